# revision 3
# baseline (speedup 1.0000x reference)
"""Trainium2 Bass kernel for the binarized ConvNet (nn_ConvNet_81501299409071).

Data-parallel over batch: 8192 images -> 8 NeuronCores x 1024 images.

Device pipeline (feature-major: features on partitions, batch on free dim),
everything is a matmul against exactly-representable +-1 Toeplitz weight
matrices; the DoReFa binarization scale E is folded into the post-matmul
activation ops (relu(acc*E + b)).

  conv1 5x5 (1->10ch):  bf16 x, 6x2 input tiles [128=(8 rows x 16 cols),
      N=512], 4 matmuls of M=120 per tile; M packed as (row-in-pair, ch,
      col-pair) so the 2x2 maxpool+bias+relu is a 4-deep chain
      u=relu(pa0+b) (ACT) then three scalar_tensor_tensor folds (DVE).
  conv2 3x3 (10->20ch): per output row, 3 accumulating K=120 matmuls x 2
      output-channel halves (fp32r).
  fc1 2000->50: 20 accumulating K=100 matmuls (one per conv2 relu tile).
  fc2 50->10 + log_softmax: exp/ln on ACT, partition sum / broadcast via
      tiny ones-matmuls, final subtract on DVE.

Startup: conv1 weights (bf16) + biases land via small leading DMAs on the
gpsimd queue; rhs DMAs stream on sync; a short burst of dummy matmuls on a
zeroed scratch tile warms the PE HAM clock-gate during the initial DMA wait.
"""
import os
import numpy as np

import concourse.bass as bass
import concourse.tile as tile
from concourse import bacc, mybir
from concourse.bass_utils import run_bass_kernel_spmd

F32 = mybir.dt.float32
F32R = mybir.dt.float32r
BF16 = mybir.dt.bfloat16

N_CORES = 8
B_TOTAL = 8192
BC = B_TOTAL // N_CORES  # 1024 images per core
N = 512                  # batch tile (free dim / PSUM bank)
N_TILES = BC // N

LAST_EXEC_TIME_NS = None
LAST_RESULTS = None

# ---------------------------------------------------------------------------
# bf16 weight blob: conv1 Toeplitz blocks (loads first, tiny)
# ---------------------------------------------------------------------------
C16_LHST1 = [[128 * (2 * j2 + par) for par in range(2)] for j2 in range(2)]
WB16_COLS = 512

# ---------------------------------------------------------------------------
# f32 weight blob: biases first (needed by the first pool ops), then conv2,
# fc1, fc2, ones — ordered so DMA chunks arrive in the order the pipeline
# needs them.
# ---------------------------------------------------------------------------
_off = 0
def _take(n):
    global _off
    c = _off
    _off += n
    return c

C_B1 = _take(1)               # [128,1]
C_B2 = [_take(1) for _s in range(2)]  # [100,1] each
CHUNK_A = _off                # biases chunk (tiny, first)
C_LHST2 = [[[[_take(100) for _h in range(2)] for _s in range(2)]
            for _pi in range(2)] for _pyo in range(2)]  # [yo%2][pair][s][h]
CHUNK_B = _off                # + conv2 weights
C_LHSTF1 = [[_take(50) for _s in range(2)] for _yo in range(10)]    # [yo][s]
C_LHSTF2 = _take(10)          # K=50
C_ONES_ROW = _take(10)        # [1,10] ones (broadcast lhsT)
C_ONES_COL = _take(1)         # [10,1] ones (partition-sum lhsT)
C_BF1 = _take(1)              # [50,1]
C_BF2 = _take(1)              # [10,1]
WBLOB_COLS = _off


def _host_prep(inputs):
    """Binarize weights, build +-1 Toeplitz matrices + bias columns packed
    into the blobs, and the E scales."""
    w1, b1 = inputs["w1"], inputs["b1"]
    w2, b2 = inputs["w2"], inputs["b2"]
    fw1, fb1 = inputs["fw1"], inputs["fb1"]
    fw2, fb2 = inputs["fw2"], inputs["fb2"]

    scales = {
        "E1": float(np.mean(np.abs(w1))),
        "E2": float(np.mean(np.abs(w2))),
        "Ef1": float(np.mean(np.abs(fw1))),
        "Ef2": float(np.mean(np.abs(fw2))),
    }
    s1 = np.sign(w1).astype(np.float32)
    s2 = np.sign(w2).astype(np.float32)
    sf1 = np.sign(fw1).astype(np.float32)
    sf2 = np.sign(fw2).astype(np.float32)

    # bf16 conv1 blob: [j2][par], K p = r*16 + xi, M m = jp*64 + oc*6 + c
    wb16 = np.zeros((128, WB16_COLS), np.float32)
    for j2 in range(2):
        for par in range(2):
            co = C16_LHST1[j2][par]
            for jp in range(2):
                j = 2 * jp + j2
                for oc in range(10):
                    for c in range(6):
                        m = jp * 64 + oc * 6 + c
                        xo = 2 * c + par
                        for dy in range(5):
                            for dx in range(5):
                                wb16[(j + dy) * 16 + xo + dx, co + m] = \
                                    s1[oc, 0, dy, dx]

    wb = np.zeros((128, WBLOB_COLS), np.float32)

    # conv2 Toeplitz [yo%2][pair pi][s][h]: K p = rp*64 + ci*6 + c where rp
    # is row-in-pair of pooled pair floor(yo/2)+pi; M m = oci*10 + xo
    for pyo in range(2):
        for pi in range(2):
            for s_ in range(2):
                for h in range(2):
                    blk = np.zeros((128, 100), np.float32)
                    for rp in range(2):
                        dy = 2 * pi + rp - pyo
                        if not (0 <= dy <= 2):
                            continue
                        for ci in range(10):
                            for c in range(6):
                                pp = rp * 64 + ci * 6 + c
                                xi = 6 * h + c
                                for oci in range(10):
                                    for xo in range(10):
                                        dx = xi - xo
                                        if 0 <= dx < 3:
                                            blk[pp, oci * 10 + xo] = \
                                                s2[10 * s_ + oci, ci, dy, dx]
                    co = C_LHST2[pyo][pi][s_][h]
                    wb[:, co:co + 100] = blk

    # fc1 [yo][s]: K p = oci*10+xo -> f = (10s+oci)*100 + yo*10 + xo
    for yo in range(10):
        for s in range(2):
            blk = np.zeros((100, 50), np.float32)
            for oci in range(10):
                for xo in range(10):
                    f = (10 * s + oci) * 100 + yo * 10 + xo
                    blk[oci * 10 + xo, :] = sf1[:, f]
            co = C_LHSTF1[yo][s]
            wb[0:100, co:co + 50] = blk

    wb[0:50, C_LHSTF2:C_LHSTF2 + 10] = sf2.T
    wb[0, C_ONES_ROW:C_ONES_ROW + 10] = 1.0
    wb[0:10, C_ONES_COL] = 1.0

    # bias columns, pre-divided by the accumulated binarization scales so
    # every bias+relu runs unscaled (relu(acc + b')) on any engine; the one
    # true scale Etot is applied at the logits.
    E1, E2, Ef1 = scales["E1"], scales["E2"], scales["Ef1"]
    b1v = np.zeros(128, np.float32)
    for jp in range(2):
        for ci in range(10):
            b1v[jp * 64 + ci * 6:jp * 64 + ci * 6 + 6] = b1[ci] / E1
    wb[:, C_B1] = b1v
    for s in range(2):
        b2v = np.repeat(b2[10 * s:10 * s + 10], 10).astype(np.float32)
        wb[0:100, C_B2[s]] = b2v / (E1 * E2)
    wb[0:50, C_BF1] = fb1 / (E1 * E2 * Ef1)
    wb[0:10, C_BF2] = fb2
    return wb16, wb, scales


# tuning knobs (engine splits / pool sizing), overridable for sweeps
CFG = {
    "n_warm": 4,          # dummy warmup MMs to heat the PE clock gate
    "rhs_bufs": 10,
    "p1_bufs": 2,
    "a2_bufs": 4,
    "pending_lag": 2,     # fc1 matmul lag behind a2 relu
    "a2_dve_mod": 4,      # a2 relu -> DVE when idx % mod == mod-1
    "u_dve_mod": 0,       # u op -> DVE when idx % mod == mod-1 (0 = never)
}


def build_program(scales, n_tiles=N_TILES, bc=BC, cfg=None, repeat=1):
    """Build the single-core SPMD bass program."""
    cfg = {**CFG, **(cfg or {})}
    Etot = scales["E1"] * scales["E2"] * scales["Ef1"] * scales["Ef2"]
    Relu = mybir.ActivationFunctionType.Relu
    Exp = mybir.ActivationFunctionType.Exp
    Ln = mybir.ActivationFunctionType.Ln
    Add = mybir.AluOpType.add
    Max = mybir.AluOpType.max
    Mult = mybir.AluOpType.mult

    nc = bacc.Bacc("TRN2", target_bir_lowering=False, debug=False)
    xT = nc.dram_tensor("xT", [28, 28, bc], BF16, kind="ExternalInput").ap()
    wblob16 = nc.dram_tensor("wblob16", [128, WB16_COLS], BF16,
                             kind="ExternalInput").ap()
    # declared float32r so fp32r matmuls may consume them directly (walrus
    # requires fp32r operands to come from fp32r-emitting producers); host
    # supplies plain fp32 bits
    wblob = nc.dram_tensor("wblob", [128, WBLOB_COLS], F32R,
                           kind="ExternalInput").ap()
    out = nc.dram_tensor("out", [10, bc], F32, kind="ExternalOutput").ap()

    with tile.TileContext(nc) as tc:
        with tc.tile_pool(name="wpool", bufs=1) as wpool, \
             tc.tile_pool(name="sb", bufs=1) as sb, \
             tc.tile_pool(name="ps", bufs=1, space="PSUM") as ps:

            wb16 = wpool.tile([128, WB16_COLS], BF16, tag="wb16")
            wb = wpool.tile([128, WBLOB_COLS], F32R, tag="wb")
            # weight blobs on the gpsimd queue: conv1(bf16) + biases first
            # (small), then conv2, then the fc tail weights
            nc.gpsimd.dma_start(wb16[:], wblob16[:])
            nc.gpsimd.dma_start(wb[:, 0:CHUNK_A], wblob[:, 0:CHUNK_A])
            nc.gpsimd.dma_start(wb[:, CHUNK_A:CHUNK_B],
                                wblob[:, CHUNK_A:CHUNK_B])
            nc.gpsimd.dma_start(wb[:, CHUNK_B:WBLOB_COLS],
                                wblob[:, CHUNK_B:WBLOB_COLS])

            # HAM warmup: dummy matmuls on a zeroed scratch tile keep the PE
            # busy while the first real rhs tiles stream in
            if cfg["n_warm"]:
                warm = sb.tile([128, N], BF16, tag="warm")
                nc.vector.memzero(warm[:])
                wps = ps.tile([128, N], F32, tag="ptail", bufs=1,
                              name="warm_ps")
                for wi in range(cfg["n_warm"]):
                    nc.tensor.matmul(wps[:], warm[:, 0:128], warm[:],
                                     start=True, stop=True)

            def wr(p0, p1, c0, c1):  # f32r slice of the weight blob
                return wb[p0:p1, c0:c1]

            b1col = wb[0:128, C_B1:C_B1 + 1].bitcast(F32)
            b2col = [wb[0:100, C_B2[s]:C_B2[s] + 1].bitcast(F32)
                     for s in range(2)]
            bf1col = wb[0:50, C_BF1:C_BF1 + 1].bitcast(F32)
            bf2col = wb[0:10, C_BF2:C_BF2 + 1].bitcast(F32)

            # per-N-tile stage emitters -----------------------------------
            def conv1_stage(nt):
                """conv1 + 2x2 maxpool (bias+relu fused) -> 12 row-pair
                tiles r2[(t, h)] with partitions (row-in-pair, ch, col)."""
                n0 = nt * N
                r2 = {}
                for q in range(6):
                    for hh in range(2):
                        r2[q, hh] = sb.tile([128, N], F32R,
                                            tag=f"r2_{q}_{hh}", bufs=2,
                                            name=f"r2_{q}_{hh}_{nt}")
                ei = 0
                for t in range(6):
                    for h in range(2):
                        rhs = sb.tile([128, N], BF16, tag="rhs1",
                                      bufs=cfg["rhs_bufs"])
                        nc.sync.dma_start(
                            rhs[:], xT[4 * t:4 * t + 8, 12 * h:12 * h + 16,
                                       n0:n0 + N])
                        pt = []
                        for j2 in range(2):
                            pa = ps.tile([128, N], F32, tag="p1e",
                                         bufs=cfg["p1_bufs"],
                                         name=f"p1e_{nt}_{t}_{h}_{j2}")
                            pb = ps.tile([128, N], F32, tag="p1o",
                                         bufs=cfg["p1_bufs"],
                                         name=f"p1o_{nt}_{t}_{h}_{j2}")
                            for par, p_ in ((0, pa), (1, pb)):
                                co = C16_LHST1[j2][par]
                                nc.tensor.matmul(p_[:],
                                                 wb16[:, co:co + 128],
                                                 rhs[:], start=True, stop=True)
                            pt += [pa, pb]
                        # 4-deep chain: u = relu(pa0+b) on ACT, then three
                        # stt folds on DVE; last writes the r2 tile
                        u = sb.tile([128, N], F32, tag="u1", bufs=3)
                        if cfg["u_dve_mod"] and \
                                ei % cfg["u_dve_mod"] == cfg["u_dve_mod"] - 1:
                            nc.vector.tensor_scalar(u[:], pt[0][:], b1col,
                                                    0.0, Add, Max)
                        else:
                            nc.scalar.activation(u[:], pt[0][:], Relu,
                                                 bias=b1col)
                        w1 = sb.tile([128, N], F32, tag="w1", bufs=3)
                        nc.vector.scalar_tensor_tensor(w1[:], pt[1][:], b1col,
                                                       u[:], Add, Max)
                        w2 = sb.tile([128, N], F32, tag="w2", bufs=3)
                        nc.vector.scalar_tensor_tensor(w2[:], pt[2][:], b1col,
                                                       w1[:], Add, Max)
                        nc.vector.scalar_tensor_tensor(r2[t, h][:], pt[3][:],
                                                       b1col, w2[:], Add, Max)
                        ei += 1
                return r2

            def conv2_fc1_stage(nt, r2):
                """conv2 + relu + fc1 accumulation (fc1 lags so the PE
                never waits on the relu engine)."""
                pfc1 = ps.tile([50, N], F32, tag="pfc1", bufs=1,
                               name=f"pfc1_{nt}")
                pending = []  # (a2_tile, fc1_col) awaiting fc1 matmul
                gi = 0
                ei = 0
                for yo in range(10):
                    for s in range(2):
                        p2 = ps.tile([100, N], F32, tag="p2", bufs=2,
                                     name=f"p2_{nt}_{yo}_{s}")
                        mi = 0
                        for pi in range(2):
                            for h in range(2):
                                co = C_LHST2[yo % 2][pi][s][h]
                                nc.tensor.matmul(
                                    p2[:], wr(0, 128, co, co + 100),
                                    r2[yo // 2 + pi, h][:],
                                    start=(mi == 0), stop=(mi == 3))
                                mi += 1
                        a2 = sb.tile([100, N], F32R, tag="a2",
                                     bufs=cfg["a2_bufs"],
                                     name=f"a2_{nt}_{yo}_{s}")
                        if ei % cfg["a2_dve_mod"] == cfg["a2_dve_mod"] - 1:
                            nc.vector.tensor_scalar(a2[:], p2[:], b2col[s],
                                                    0.0, Add, Max)
                        else:
                            nc.scalar.activation(a2[:], p2[:], Relu,
                                                 bias=b2col[s])
                        ei += 1
                        pending.append((a2, C_LHSTF1[yo][s]))
                        if len(pending) > cfg["pending_lag"]:
                            pa2, pcf = pending.pop(0)
                            nc.tensor.matmul(
                                pfc1[:], wr(0, 100, pcf, pcf + 50),
                                pa2[:],
                                start=(gi == 0), stop=False)
                            gi += 1
                while pending:
                    pa2, pcf = pending.pop(0)
                    nc.tensor.matmul(pfc1[:], wr(0, 100, pcf, pcf + 50),
                                     pa2[:], start=(gi == 0),
                                     stop=(len(pending) == 0))
                    gi += 1
                return pfc1

            def tail_stage(nt, pfc1):
                """fc2 + log_softmax + output DMA."""
                n0 = nt * N
                a3 = sb.tile([50, N], F32R, tag="a3", bufs=2,
                             name=f"a3_{nt}")
                nc.scalar.activation(a3[:], pfc1[:], Relu, bias=bf1col)
                zps = ps.tile([10, N], F32, tag="ptail", bufs=1,
                              name=f"zps_{nt}")
                nc.tensor.matmul(zps[:], wr(0, 50, C_LHSTF2, C_LHSTF2 + 10),
                                 a3[:], start=True, stop=True)
                z = sb.tile([10, N], F32, tag="z", bufs=2, name=f"z_{nt}")
                nc.vector.tensor_scalar(z[:], zps[:], Etot, bf2col,
                                        Mult, Add)
                ez = sb.tile([10, N], F32R, tag="ez", bufs=2,
                             name=f"ez_{nt}")
                nc.scalar.activation(ez[:], zps[:], Exp, bias=bf2col,
                                     scale=Etot)
                sps = ps.tile([1, N], F32, tag="ptail", bufs=1,
                               name=f"sps_{nt}")
                nc.tensor.matmul(sps[:], wr(0, 10, C_ONES_COL, C_ONES_COL + 1),
                                 ez[:], start=True, stop=True)
                lse = sb.tile([1, N], F32R, tag="lse", bufs=2,
                              name=f"lse_{nt}")
                nc.scalar.activation(lse[:], sps[:], Ln)
                bps = ps.tile([10, N], F32, tag="ptail", bufs=1,
                               name=f"bps_{nt}")
                nc.tensor.matmul(bps[:], wr(0, 1, C_ONES_ROW, C_ONES_ROW + 10),
                                 lse[:], start=True, stop=True)
                osb = sb.tile([10, N], F32, tag="osb", bufs=2, name=f"osb_{nt}")
                nc.vector.tensor_sub(osb[:], z[:], bps[:])
                nc.sync.dma_start(out[:, n0:n0 + N], osb[:])

            # interleave N-tiles: tile k+1's conv1 is emitted before tile
            # k's tail so the PE stays dense across the serial softmax tail
            for _rep in range(repeat):
                r2s = {}
                for nt in range(n_tiles):
                    r2s[nt] = conv1_stage(nt)
                    if nt > 0:
                        k = nt - 1
                        tail_stage(k, conv2_fc1_stage(k, r2s.pop(k)))
                k = n_tiles - 1
                tail_stage(k, conv2_fc1_stage(k, r2s.pop(k)))
    nc.compile()
    return nc


def kernel(**inputs):
    global LAST_EXEC_TIME_NS, LAST_RESULTS
    x = np.ascontiguousarray(np.asarray(inputs["x"], dtype=np.float32))
    wb16, wb, scales = _host_prep({k: np.asarray(v) for k, v in inputs.items()})

    nc = build_program(scales)

    import ml_dtypes
    wb16_b = wb16.astype(ml_dtypes.bfloat16)
    in_maps = []
    for i in range(N_CORES):
        xs = x[i * BC:(i + 1) * BC, 0]            # [BC, 28, 28]
        xTi = np.ascontiguousarray(
            xs.transpose(1, 2, 0).astype(ml_dtypes.bfloat16))  # [28, 28, BC]
        in_maps.append({"xT": xTi, "wblob16": wb16_b, "wblob": wb})

    trace = bool(os.environ.get("KERNEL_TRACE"))
    res = run_bass_kernel_spmd(nc, in_maps, list(range(N_CORES)), trace=trace)
    LAST_EXEC_TIME_NS = res.exec_time_ns
    LAST_RESULTS = res

    out = np.empty((B_TOTAL, 10), np.float32)
    for i in range(N_CORES):
        out[i * BC:(i + 1) * BC] = res.results[i]["out"].T
    return out


# revision 4
# speedup vs baseline: 1.0258x; 1.0258x over previous
"""Trainium2 Bass kernel for the binarized ConvNet (nn_ConvNet_81501299409071).

Data-parallel over batch: 8192 images -> 8 NeuronCores x 1024 images.

Device pipeline (feature-major: features on partitions, batch on free dim),
everything is a matmul against exactly-representable +-1 Toeplitz weight
matrices; the DoReFa binarization scale E is folded into the post-matmul
activation ops (relu(acc*E + b)).

  conv1 5x5 (1->10ch):  bf16 x, 6x2 input tiles [128=(8 rows x 16 cols),
      N=512], 4 matmuls of M=120 per tile; M packed as (row-in-pair, ch,
      col-pair) so the 2x2 maxpool+bias+relu is a 4-deep chain
      u=relu(pa0+b) (ACT) then three scalar_tensor_tensor folds (DVE).
  conv2 3x3 (10->20ch): per output row, 3 accumulating K=120 matmuls x 2
      output-channel halves (fp32r).
  fc1 2000->50: 20 accumulating K=100 matmuls (one per conv2 relu tile).
  fc2 50->10 + log_softmax: exp/ln on ACT, partition sum / broadcast via
      tiny ones-matmuls, final subtract on DVE.

Startup: conv1 weights (bf16) + biases land via small leading DMAs on the
gpsimd queue; rhs DMAs stream on sync; a short burst of dummy matmuls on a
zeroed scratch tile warms the PE HAM clock-gate during the initial DMA wait.
"""
import os
import numpy as np

import concourse.bass as bass
import concourse.tile as tile
from concourse import bacc, mybir
from concourse.bass_utils import run_bass_kernel_spmd

F32 = mybir.dt.float32
F32R = mybir.dt.float32r
BF16 = mybir.dt.bfloat16

N_CORES = 8
B_TOTAL = 8192
BC = B_TOTAL // N_CORES  # 1024 images per core
N = 512                  # batch tile (free dim / PSUM bank)
N_TILES = BC // N

LAST_EXEC_TIME_NS = None
LAST_RESULTS = None

# ---------------------------------------------------------------------------
# bf16 weight blob: conv1 Toeplitz blocks (loads first, tiny)
# ---------------------------------------------------------------------------
C16_LHST1 = [[128 * (2 * j2 + par) for par in range(2)] for j2 in range(2)]
WB16_COLS = 512

# ---------------------------------------------------------------------------
# f32 weight blob: biases first (needed by the first pool ops), then conv2,
# fc1, fc2, ones — ordered so DMA chunks arrive in the order the pipeline
# needs them.
# ---------------------------------------------------------------------------
_off = 0
def _take(n):
    global _off
    c = _off
    _off += n
    return c

C_B1 = _take(1)               # [128,1]
C_B2 = [_take(1) for _s in range(2)]  # [100,1] each
CHUNK_A = _off                # biases chunk (tiny, first)
C_LHST2 = [[[[_take(100) for _h in range(2)] for _s in range(2)]
            for _pi in range(2)] for _pyo in range(2)]  # [yo%2][pair][s][h]
CHUNK_B = _off                # + conv2 weights
C_LHSTF1 = [[_take(50) for _s in range(2)] for _yo in range(10)]    # [yo][s]
C_LHSTF2 = _take(10)          # K=50
C_ONES_ROW = _take(10)        # [1,10] ones (broadcast lhsT)
C_ONES_COL = _take(1)         # [10,1] ones (partition-sum lhsT)
C_BF1 = _take(1)              # [50,1]
C_BF2 = _take(1)              # [10,1]
WBLOB_COLS = _off


def _host_prep(inputs):
    """Binarize weights, build +-1 Toeplitz matrices + bias columns packed
    into the blobs, and the E scales."""
    w1, b1 = inputs["w1"], inputs["b1"]
    w2, b2 = inputs["w2"], inputs["b2"]
    fw1, fb1 = inputs["fw1"], inputs["fb1"]
    fw2, fb2 = inputs["fw2"], inputs["fb2"]

    scales = {
        "E1": float(np.mean(np.abs(w1))),
        "E2": float(np.mean(np.abs(w2))),
        "Ef1": float(np.mean(np.abs(fw1))),
        "Ef2": float(np.mean(np.abs(fw2))),
    }
    s1 = np.sign(w1).astype(np.float32)
    s2 = np.sign(w2).astype(np.float32)
    sf1 = np.sign(fw1).astype(np.float32)
    sf2 = np.sign(fw2).astype(np.float32)

    # bf16 conv1 blob: [j2][par], K p = r*16 + xi, M m = jp*64 + oc*6 + c
    wb16 = np.zeros((128, WB16_COLS), np.float32)
    for j2 in range(2):
        for par in range(2):
            co = C16_LHST1[j2][par]
            for jp in range(2):
                j = 2 * jp + j2
                for oc in range(10):
                    for c in range(6):
                        m = jp * 64 + oc * 6 + c
                        xo = 2 * c + par
                        for dy in range(5):
                            for dx in range(5):
                                wb16[(j + dy) * 16 + xo + dx, co + m] = \
                                    s1[oc, 0, dy, dx]

    wb = np.zeros((128, WBLOB_COLS), np.float32)

    # conv2 Toeplitz [yo%2][pair pi][s][h]: K p = rp*64 + ci*6 + c where rp
    # is row-in-pair of pooled pair floor(yo/2)+pi; M m = oci*10 + xo
    for pyo in range(2):
        for pi in range(2):
            for s_ in range(2):
                for h in range(2):
                    blk = np.zeros((128, 100), np.float32)
                    for rp in range(2):
                        dy = 2 * pi + rp - pyo
                        if not (0 <= dy <= 2):
                            continue
                        for ci in range(10):
                            for c in range(6):
                                pp = rp * 64 + ci * 6 + c
                                xi = 6 * h + c
                                for oci in range(10):
                                    for xo in range(10):
                                        dx = xi - xo
                                        if 0 <= dx < 3:
                                            blk[pp, oci * 10 + xo] = \
                                                s2[10 * s_ + oci, ci, dy, dx]
                    co = C_LHST2[pyo][pi][s_][h]
                    wb[:, co:co + 100] = blk

    # fc1 [yo][s]: K p = oci*10+xo -> f = (10s+oci)*100 + yo*10 + xo
    for yo in range(10):
        for s in range(2):
            blk = np.zeros((100, 50), np.float32)
            for oci in range(10):
                for xo in range(10):
                    f = (10 * s + oci) * 100 + yo * 10 + xo
                    blk[oci * 10 + xo, :] = sf1[:, f]
            co = C_LHSTF1[yo][s]
            wb[0:100, co:co + 50] = blk

    wb[0:50, C_LHSTF2:C_LHSTF2 + 10] = sf2.T
    wb[0, C_ONES_ROW:C_ONES_ROW + 10] = 1.0
    wb[0:10, C_ONES_COL] = 1.0

    # bias columns, pre-divided by the accumulated binarization scales so
    # every bias+relu runs unscaled (relu(acc + b')) on any engine; the one
    # true scale Etot is applied at the logits.
    E1, E2, Ef1 = scales["E1"], scales["E2"], scales["Ef1"]
    b1v = np.zeros(128, np.float32)
    for jp in range(2):
        for ci in range(10):
            b1v[jp * 64 + ci * 6:jp * 64 + ci * 6 + 6] = b1[ci] / E1
    wb[:, C_B1] = b1v
    for s in range(2):
        b2v = np.repeat(b2[10 * s:10 * s + 10], 10).astype(np.float32)
        wb[0:100, C_B2[s]] = b2v / (E1 * E2)
    wb[0:50, C_BF1] = fb1 / (E1 * E2 * Ef1)
    wb[0:10, C_BF2] = fb2
    return wb16, wb, scales


# tuning knobs (engine splits / pool sizing), overridable for sweeps
CFG = {
    "n_warm": 4,          # dummy warmup MMs to heat the PE clock gate
    "rhs_bufs": 10,
    "p1_bufs": 2,
    "a2_bufs": 4,
    "pending_lag": 2,     # fc1 matmul lag behind a2 relu
    "a2_dve_mod": 4,      # a2 relu -> DVE when idx % mod == mod-1
    "u_dve_mod": 0,       # u op -> DVE when idx % mod == mod-1 (0 = never)
}


def build_program(scales, n_tiles=N_TILES, bc=BC, cfg=None, repeat=1):
    """Build the single-core SPMD bass program."""
    cfg = {**CFG, **(cfg or {})}
    Etot = scales["E1"] * scales["E2"] * scales["Ef1"] * scales["Ef2"]
    Relu = mybir.ActivationFunctionType.Relu
    Exp = mybir.ActivationFunctionType.Exp
    Ln = mybir.ActivationFunctionType.Ln
    Add = mybir.AluOpType.add
    Max = mybir.AluOpType.max
    Mult = mybir.AluOpType.mult

    nc = bacc.Bacc("TRN2", target_bir_lowering=False, debug=False)
    xT = nc.dram_tensor("xT", [28, 28, bc], BF16, kind="ExternalInput").ap()
    wblob16 = nc.dram_tensor("wblob16", [128, WB16_COLS], BF16,
                             kind="ExternalInput").ap()
    # declared float32r so fp32r matmuls may consume them directly (walrus
    # requires fp32r operands to come from fp32r-emitting producers); host
    # supplies plain fp32 bits
    wblob = nc.dram_tensor("wblob", [128, WBLOB_COLS], F32R,
                           kind="ExternalInput").ap()
    out = nc.dram_tensor("out", [10, bc], F32, kind="ExternalOutput").ap()

    with tile.TileContext(nc) as tc:
        with tc.tile_pool(name="wpool", bufs=1) as wpool, \
             tc.tile_pool(name="sb", bufs=1) as sb, \
             tc.tile_pool(name="ps", bufs=1, space="PSUM") as ps:

            wb16 = wpool.tile([128, WB16_COLS], BF16, tag="wb16")
            wb = wpool.tile([128, WBLOB_COLS], F32R, tag="wb")
            # weight blobs on the gpsimd queue: conv1(bf16) + biases first
            # (small), then conv2, then the fc tail weights
            nc.gpsimd.dma_start(wb16[:], wblob16[:])
            nc.gpsimd.dma_start(wb[:, 0:CHUNK_A], wblob[:, 0:CHUNK_A])
            nc.gpsimd.dma_start(wb[:, CHUNK_A:CHUNK_B],
                                wblob[:, CHUNK_A:CHUNK_B])
            nc.gpsimd.dma_start(wb[:, CHUNK_B:WBLOB_COLS],
                                wblob[:, CHUNK_B:WBLOB_COLS])

            # HAM warmup: dummy matmuls on a zeroed scratch tile keep the PE
            # busy while the first real rhs tiles stream in
            if cfg["n_warm"]:
                warm = sb.tile([128, N], BF16, tag="warm")
                nc.vector.memzero(warm[:])
                wps = ps.tile([128, N], F32, tag="ptail", bufs=1,
                              name="warm_ps")
                for wi in range(cfg["n_warm"]):
                    nc.tensor.matmul(wps[:], warm[:, 0:128], warm[:],
                                     start=True, stop=True)

            def wr(p0, p1, c0, c1):  # f32r slice of the weight blob
                return wb[p0:p1, c0:c1]

            b1col = wb[0:128, C_B1:C_B1 + 1].bitcast(F32)
            b2col = [wb[0:100, C_B2[s]:C_B2[s] + 1].bitcast(F32)
                     for s in range(2)]
            bf1col = wb[0:50, C_BF1:C_BF1 + 1].bitcast(F32)
            bf2col = wb[0:10, C_BF2:C_BF2 + 1].bitcast(F32)

            # per-N-tile stage emitters -----------------------------------
            def conv1_stage(nt):
                """conv1 + 2x2 maxpool (bias+relu fused) -> 12 row-pair
                tiles r2[(t, h)] with partitions (row-in-pair, ch, col)."""
                n0 = nt * N
                r2 = {}
                for q in range(6):
                    for hh in range(2):
                        r2[q, hh] = sb.tile([128, N], F32R,
                                            tag=f"r2_{q}_{hh}", bufs=2,
                                            name=f"r2_{q}_{hh}_{nt}")
                ei = 0
                for t in range(6):
                    for h in range(2):
                        rhs = sb.tile([128, N], BF16, tag="rhs1",
                                      bufs=cfg["rhs_bufs"])
                        nc.sync.dma_start(
                            rhs[:], xT[4 * t:4 * t + 8, 12 * h:12 * h + 16,
                                       n0:n0 + N])
                        V = []
                        for j2 in range(2):
                            pa = ps.tile([128, N], F32, tag="p1e",
                                         bufs=cfg["p1_bufs"],
                                         name=f"p1e_{nt}_{t}_{h}_{j2}")
                            pb = ps.tile([128, N], F32, tag="p1o",
                                         bufs=cfg["p1_bufs"],
                                         name=f"p1o_{nt}_{t}_{h}_{j2}")
                            for par, p_ in ((0, pa), (1, pb)):
                                co = C16_LHST1[j2][par]
                                nc.tensor.matmul(p_[:],
                                                 wb16[:, co:co + 128],
                                                 rhs[:], start=True, stop=True)
                            # u = relu(Pa + b): the relu makes the final
                            # plain max-fold equal relu(pool+b)
                            u = sb.tile([128, N], F32, tag="u1", bufs=3)
                            if cfg["u_dve_mod"] and \
                                    ei % cfg["u_dve_mod"] == \
                                    cfg["u_dve_mod"] - 1:
                                nc.vector.tensor_scalar(u[:], pa[:], b1col,
                                                        0.0, Add, Max)
                            else:
                                nc.scalar.activation(u[:], pa[:], Relu,
                                                     bias=b1col)
                            # v = max(Pb + b, u): column pool on DVE
                            v = sb.tile([128, N], F32, tag=f"v1_{j2}", bufs=2)
                            nc.vector.scalar_tensor_tensor(
                                v[:], pb[:], b1col, u[:], Add, Max)
                            V.append(v)
                            ei += 1
                        # row pool (relu already folded into u): equal-base
                        nc.vector.tensor_max(r2[t, h][:], V[0][:], V[1][:])
                return r2

            def conv2_fc1_stage(nt, r2):
                """conv2 + relu + fc1 accumulation (fc1 lags so the PE
                never waits on the relu engine)."""
                pfc1 = ps.tile([50, N], F32, tag="pfc1", bufs=1,
                               name=f"pfc1_{nt}")
                pending = []  # (a2_tile, fc1_col) awaiting fc1 matmul
                gi = 0
                ei = 0
                for yo in range(10):
                    for s in range(2):
                        p2 = ps.tile([100, N], F32, tag="p2", bufs=2,
                                     name=f"p2_{nt}_{yo}_{s}")
                        mi = 0
                        for pi in range(2):
                            for h in range(2):
                                co = C_LHST2[yo % 2][pi][s][h]
                                nc.tensor.matmul(
                                    p2[:], wr(0, 128, co, co + 100),
                                    r2[yo // 2 + pi, h][:],
                                    start=(mi == 0), stop=(mi == 3))
                                mi += 1
                        a2 = sb.tile([100, N], F32R, tag="a2",
                                     bufs=cfg["a2_bufs"],
                                     name=f"a2_{nt}_{yo}_{s}")
                        if ei % cfg["a2_dve_mod"] == cfg["a2_dve_mod"] - 1:
                            nc.vector.tensor_scalar(a2[:], p2[:], b2col[s],
                                                    0.0, Add, Max)
                        else:
                            nc.scalar.activation(a2[:], p2[:], Relu,
                                                 bias=b2col[s])
                        ei += 1
                        pending.append((a2, C_LHSTF1[yo][s]))
                        if len(pending) > cfg["pending_lag"]:
                            pa2, pcf = pending.pop(0)
                            nc.tensor.matmul(
                                pfc1[:], wr(0, 100, pcf, pcf + 50),
                                pa2[:],
                                start=(gi == 0), stop=False)
                            gi += 1
                while pending:
                    pa2, pcf = pending.pop(0)
                    nc.tensor.matmul(pfc1[:], wr(0, 100, pcf, pcf + 50),
                                     pa2[:], start=(gi == 0),
                                     stop=(len(pending) == 0))
                    gi += 1
                return pfc1

            def tail_stage(nt, pfc1):
                """fc2 + log_softmax + output DMA."""
                n0 = nt * N
                a3 = sb.tile([50, N], F32R, tag="a3", bufs=2,
                             name=f"a3_{nt}")
                nc.scalar.activation(a3[:], pfc1[:], Relu, bias=bf1col)
                zps = ps.tile([10, N], F32, tag="ptail", bufs=1,
                              name=f"zps_{nt}")
                nc.tensor.matmul(zps[:], wr(0, 50, C_LHSTF2, C_LHSTF2 + 10),
                                 a3[:], start=True, stop=True)
                z = sb.tile([10, N], F32, tag="z", bufs=2, name=f"z_{nt}")
                nc.vector.tensor_scalar(z[:], zps[:], Etot, bf2col,
                                        Mult, Add)
                ez = sb.tile([10, N], F32R, tag="ez", bufs=2,
                             name=f"ez_{nt}")
                nc.scalar.activation(ez[:], zps[:], Exp, bias=bf2col,
                                     scale=Etot)
                sps = ps.tile([1, N], F32, tag="ptail", bufs=1,
                               name=f"sps_{nt}")
                nc.tensor.matmul(sps[:], wr(0, 10, C_ONES_COL, C_ONES_COL + 1),
                                 ez[:], start=True, stop=True)
                lse = sb.tile([1, N], F32R, tag="lse", bufs=2,
                              name=f"lse_{nt}")
                nc.scalar.activation(lse[:], sps[:], Ln)
                bps = ps.tile([10, N], F32, tag="ptail", bufs=1,
                               name=f"bps_{nt}")
                nc.tensor.matmul(bps[:], wr(0, 1, C_ONES_ROW, C_ONES_ROW + 10),
                                 lse[:], start=True, stop=True)
                osb = sb.tile([10, N], F32, tag="osb", bufs=2, name=f"osb_{nt}")
                nc.vector.tensor_sub(osb[:], z[:], bps[:])
                nc.sync.dma_start(out[:, n0:n0 + N], osb[:])

            # interleave N-tiles: tile k+1's conv1 is emitted before tile
            # k's tail so the PE stays dense across the serial softmax tail
            for _rep in range(repeat):
                r2s = {}
                for nt in range(n_tiles):
                    r2s[nt] = conv1_stage(nt)
                    if nt > 0:
                        k = nt - 1
                        tail_stage(k, conv2_fc1_stage(k, r2s.pop(k)))
                k = n_tiles - 1
                tail_stage(k, conv2_fc1_stage(k, r2s.pop(k)))
    nc.compile()
    return nc


def kernel(**inputs):
    global LAST_EXEC_TIME_NS, LAST_RESULTS
    x = np.ascontiguousarray(np.asarray(inputs["x"], dtype=np.float32))
    wb16, wb, scales = _host_prep({k: np.asarray(v) for k, v in inputs.items()})

    nc = build_program(scales)

    import ml_dtypes
    wb16_b = wb16.astype(ml_dtypes.bfloat16)
    in_maps = []
    for i in range(N_CORES):
        xs = x[i * BC:(i + 1) * BC, 0]            # [BC, 28, 28]
        xTi = np.ascontiguousarray(
            xs.transpose(1, 2, 0).astype(ml_dtypes.bfloat16))  # [28, 28, BC]
        in_maps.append({"xT": xTi, "wblob16": wb16_b, "wblob": wb})

    trace = bool(os.environ.get("KERNEL_TRACE"))
    res = run_bass_kernel_spmd(nc, in_maps, list(range(N_CORES)), trace=trace)
    LAST_EXEC_TIME_NS = res.exec_time_ns
    LAST_RESULTS = res

    out = np.empty((B_TOTAL, 10), np.float32)
    for i in range(N_CORES):
        out[i * BC:(i + 1) * BC] = res.results[i]["out"].T
    return out


# revision 7
# speedup vs baseline: 1.1345x; 1.1059x over previous
"""Trainium2 Bass kernel for the binarized ConvNet (nn_ConvNet_81501299409071).

Data-parallel over batch: 8192 images -> 8 NeuronCores x 1024 images.

Device pipeline (feature-major: features on partitions, batch on free dim),
everything is a matmul against exactly-representable +-1 Toeplitz weight
matrices; the DoReFa binarization scale E is folded into the post-matmul
activation ops (relu(acc*E + b)).

  conv1 5x5 (1->10ch):  bf16 x, 6x2 input tiles [128=(8 rows x 16 cols),
      N=512], 4 matmuls of M=120 per tile; M packed as (row-in-pair, ch,
      col-pair) so the 2x2 maxpool+bias+relu is a 4-deep chain
      u=relu(pa0+b) (ACT) then three scalar_tensor_tensor folds (DVE).
  conv2 3x3 (10->20ch): per output row, 3 accumulating K=120 matmuls x 2
      output-channel halves (fp32r).
  fc1 2000->50: 20 accumulating K=100 matmuls (one per conv2 relu tile).
  fc2 50->10 + log_softmax: exp/ln on ACT, partition sum / broadcast via
      tiny ones-matmuls, final subtract on DVE.

Startup: conv1 weights (bf16) + biases land via small leading DMAs on the
gpsimd queue; rhs DMAs stream on sync; a short burst of dummy matmuls on a
zeroed scratch tile warms the PE HAM clock-gate during the initial DMA wait.
"""
import os
import numpy as np

import concourse.bass as bass
import concourse.tile as tile
from concourse import bacc, mybir
from concourse.bass_utils import run_bass_kernel_spmd

F32 = mybir.dt.float32
F32R = mybir.dt.float32r
BF16 = mybir.dt.bfloat16

N_CORES = 8
B_TOTAL = 8192
BC = B_TOTAL // N_CORES  # 1024 images per core
N = 512                  # batch tile (free dim / PSUM bank)
N_TILES = BC // N

LAST_EXEC_TIME_NS = None
LAST_RESULTS = None

# ---------------------------------------------------------------------------
# bf16 weight blob: conv1 Toeplitz blocks (loads first, tiny)
# ---------------------------------------------------------------------------
C16_LHST1 = [[128 * (2 * j2 + par) for par in range(2)] for j2 in range(2)]
WB16_COLS = 512

# ---------------------------------------------------------------------------
# f32 weight blob: biases first (needed by the first pool ops), then conv2,
# fc1, fc2, ones — ordered so DMA chunks arrive in the order the pipeline
# needs them.
# ---------------------------------------------------------------------------
_off = 0
def _take(n):
    global _off
    c = _off
    _off += n
    return c

C_B1 = _take(1)               # [128,1]
C_B2 = [_take(1) for _s in range(2)]  # [100,1] each
CHUNK_A = _off                # biases chunk (tiny, first)
C_LHST2 = [[[[_take(100) for _h in range(2)] for _s in range(2)]
            for _pi in range(2)] for _pyo in range(2)]  # [yo%2][pair][s][h]
CHUNK_B = _off                # + conv2 weights
C_LHSTF1 = [[_take(50) for _s in range(2)] for _yo in range(10)]    # [yo][s]
C_LHSTF2 = _take(10)          # K=50
C_ONES_ROW = _take(10)        # [1,10] ones (broadcast lhsT)
C_ONES_COL = _take(1)         # [10,1] ones (partition-sum lhsT)
C_BF1 = _take(1)              # [50,1]
C_BF2 = _take(1)              # [10,1]
WBLOB_COLS = _off


def _host_prep(inputs):
    """Binarize weights, build +-1 Toeplitz matrices + bias columns packed
    into the blobs, and the E scales."""
    w1, b1 = inputs["w1"], inputs["b1"]
    w2, b2 = inputs["w2"], inputs["b2"]
    fw1, fb1 = inputs["fw1"], inputs["fb1"]
    fw2, fb2 = inputs["fw2"], inputs["fb2"]

    scales = {
        "E1": float(np.mean(np.abs(w1))),
        "E2": float(np.mean(np.abs(w2))),
        "Ef1": float(np.mean(np.abs(fw1))),
        "Ef2": float(np.mean(np.abs(fw2))),
    }
    s1 = np.sign(w1).astype(np.float32)
    s2 = np.sign(w2).astype(np.float32)
    sf1 = np.sign(fw1).astype(np.float32)
    sf2 = np.sign(fw2).astype(np.float32)

    # bf16 conv1 blob: [j2][par], K p = r*16 + xi, M m = jp*64 + oc*6 + c
    wb16 = np.zeros((128, WB16_COLS), np.float32)
    for j2 in range(2):
        for par in range(2):
            co = C16_LHST1[j2][par]
            for jp in range(2):
                j = 2 * jp + j2
                for oc in range(10):
                    for c in range(6):
                        m = jp * 64 + oc * 6 + c
                        xo = 2 * c + par
                        for dy in range(5):
                            for dx in range(5):
                                wb16[(j + dy) * 16 + xo + dx, co + m] = \
                                    s1[oc, 0, dy, dx]

    wb = np.zeros((128, WBLOB_COLS), np.float32)

    # conv2 Toeplitz [yo%2][pair pi][s][h]: K p = rp*64 + ci*6 + c where rp
    # is row-in-pair of pooled pair floor(yo/2)+pi; M m = oci*10 + xo
    for pyo in range(2):
        for pi in range(2):
            for s_ in range(2):
                for h in range(2):
                    blk = np.zeros((128, 100), np.float32)
                    for rp in range(2):
                        dy = 2 * pi + rp - pyo
                        if not (0 <= dy <= 2):
                            continue
                        for ci in range(10):
                            for c in range(6):
                                pp = rp * 64 + ci * 6 + c
                                xi = 6 * h + c
                                for oci in range(10):
                                    for xo in range(10):
                                        dx = xi - xo
                                        if 0 <= dx < 3:
                                            blk[pp, oci * 10 + xo] = \
                                                s2[10 * s_ + oci, ci, dy, dx]
                    co = C_LHST2[pyo][pi][s_][h]
                    wb[:, co:co + 100] = blk

    # fc1 [yo][s]: K p = oci*10+xo -> f = (10s+oci)*100 + yo*10 + xo
    for yo in range(10):
        for s in range(2):
            blk = np.zeros((100, 50), np.float32)
            for oci in range(10):
                for xo in range(10):
                    f = (10 * s + oci) * 100 + yo * 10 + xo
                    blk[oci * 10 + xo, :] = sf1[:, f]
            co = C_LHSTF1[yo][s]
            wb[0:100, co:co + 50] = blk

    wb[0:50, C_LHSTF2:C_LHSTF2 + 10] = sf2.T
    wb[0, C_ONES_ROW:C_ONES_ROW + 10] = 1.0
    wb[0:10, C_ONES_COL] = 1.0

    # bias columns, pre-divided by the accumulated binarization scales so
    # every bias+relu runs unscaled (relu(acc + b')) on any engine; the one
    # true scale Etot is applied at the logits.
    E1, E2, Ef1 = scales["E1"], scales["E2"], scales["Ef1"]
    b1v = np.zeros(128, np.float32)
    for jp in range(2):
        for ci in range(10):
            b1v[jp * 64 + ci * 6:jp * 64 + ci * 6 + 6] = b1[ci] / E1
    wb[:, C_B1] = b1v
    for s in range(2):
        b2v = np.repeat(b2[10 * s:10 * s + 10], 10).astype(np.float32)
        wb[0:100, C_B2[s]] = b2v / (E1 * E2)
    wb[0:50, C_BF1] = fb1 / (E1 * E2 * Ef1)
    wb[0:10, C_BF2] = fb2
    return wb16, wb, scales


# tuning knobs (engine splits / pool sizing), overridable for sweeps
CFG = {
    "n_warm": 4,          # dummy warmup MMs to heat the PE clock gate
    "rhs_bufs": 10,
    "p1_bufs": 2,
    "a2_bufs": 4,
    "pending_lag": 2,     # fc1 matmul lag behind a2 relu
    "a2_dve_mod": 4,      # a2 relu -> DVE when idx % mod == mod-1
    "u_dve_mod": 0,       # u op -> DVE when idx % mod == mod-1 (0 = never)
}


def build_program(scales, n_tiles=N_TILES, bc=BC, cfg=None, repeat=1):
    """Build the single-core SPMD bass program."""
    cfg = {**CFG, **(cfg or {})}
    Etot = scales["E1"] * scales["E2"] * scales["Ef1"] * scales["Ef2"]
    Relu = mybir.ActivationFunctionType.Relu
    Exp = mybir.ActivationFunctionType.Exp
    Ln = mybir.ActivationFunctionType.Ln
    Add = mybir.AluOpType.add
    Max = mybir.AluOpType.max
    Mult = mybir.AluOpType.mult

    nc = bacc.Bacc("TRN2", target_bir_lowering=False, debug=False)
    xT = nc.dram_tensor("xT", [28, 28, bc], BF16, kind="ExternalInput").ap()
    wblob16 = nc.dram_tensor("wblob16", [128, WB16_COLS], BF16,
                             kind="ExternalInput").ap()
    # declared float32r so fp32r matmuls may consume them directly (walrus
    # requires fp32r operands to come from fp32r-emitting producers); host
    # supplies plain fp32 bits
    wblob = nc.dram_tensor("wblob", [128, WBLOB_COLS], F32R,
                           kind="ExternalInput").ap()
    out = nc.dram_tensor("out", [10, bc], F32, kind="ExternalOutput").ap()

    with tile.TileContext(nc) as tc:
        with tc.tile_pool(name="wpool", bufs=1) as wpool, \
             tc.tile_pool(name="sb", bufs=1) as sb, \
             tc.tile_pool(name="ps", bufs=1, space="PSUM") as ps:

            wb16 = wpool.tile([128, WB16_COLS], BF16, tag="wb16")
            wb = wpool.tile([128, WBLOB_COLS], F32R, tag="wb")
            # weight blobs on the gpsimd queue: conv1(bf16) + biases first
            # (small), then conv2, then the fc tail weights
            nc.gpsimd.dma_start(wb16[:], wblob16[:])
            nc.gpsimd.dma_start(wb[:, 0:CHUNK_A], wblob[:, 0:CHUNK_A])
            nc.gpsimd.dma_start(wb[:, CHUNK_A:CHUNK_B],
                                wblob[:, CHUNK_A:CHUNK_B])
            nc.gpsimd.dma_start(wb[:, CHUNK_B:WBLOB_COLS],
                                wblob[:, CHUNK_B:WBLOB_COLS])

            # HAM warmup: dummy matmuls on a zeroed scratch tile keep the PE
            # busy while the first real rhs tiles stream in
            if cfg["n_warm"]:
                warm = sb.tile([128, N], BF16, tag="warm")
                nc.vector.memzero(warm[:])
                wps = ps.tile([128, N], F32, tag="ptail", bufs=1,
                              name="warm_ps")
                for wi in range(cfg["n_warm"]):
                    nc.tensor.matmul(wps[:], warm[:, 0:128], warm[:],
                                     start=True, stop=True)

            def wr(p0, p1, c0, c1):  # f32r slice of the weight blob
                return wb[p0:p1, c0:c1]

            b1col = wb[0:128, C_B1:C_B1 + 1].bitcast(F32)
            b2col = [wb[0:100, C_B2[s]:C_B2[s] + 1].bitcast(F32)
                     for s in range(2)]
            bf1col = wb[0:50, C_BF1:C_BF1 + 1].bitcast(F32)
            bf2col = wb[0:10, C_BF2:C_BF2 + 1].bitcast(F32)

            # per-N-tile stage emitters -----------------------------------
            def alloc_r2(nt):
                r2 = {}
                for q in range(6):
                    for hh in range(2):
                        r2[q, hh] = sb.tile([128, N], F32R,
                                            tag=f"r2_{q}_{hh}", bufs=2,
                                            name=f"r2_{q}_{hh}_{nt}")
                return r2

            def conv1_group(nt, t, h, r2, ei):
                """One conv1 rhs tile: DMA + 4 matmuls + 2x2 maxpool chain
                (bias+relu fused) into r2[(t, h)]."""
                n0 = nt * N
                if True:
                    if True:
                        rhs = sb.tile([128, N], BF16, tag="rhs1",
                                      bufs=cfg["rhs_bufs"])
                        nc.sync.dma_start(
                            rhs[:], xT[4 * t:4 * t + 8, 12 * h:12 * h + 16,
                                       n0:n0 + N])
                        V = []
                        for j2 in range(2):
                            pa = ps.tile([128, N], F32, tag="p1e",
                                         bufs=cfg["p1_bufs"],
                                         name=f"p1e_{nt}_{t}_{h}_{j2}")
                            pb = ps.tile([128, N], F32, tag="p1o",
                                         bufs=cfg["p1_bufs"],
                                         name=f"p1o_{nt}_{t}_{h}_{j2}")
                            for par, p_ in ((0, pa), (1, pb)):
                                co = C16_LHST1[j2][par]
                                nc.tensor.matmul(p_[:],
                                                 wb16[:, co:co + 128],
                                                 rhs[:], start=True, stop=True)
                            # u = relu(Pa + b): the relu makes the final
                            # plain max-fold equal relu(pool+b)
                            u = sb.tile([128, N], F32, tag="u1", bufs=3)
                            if cfg["u_dve_mod"] and \
                                    ei % cfg["u_dve_mod"] == \
                                    cfg["u_dve_mod"] - 1:
                                nc.vector.tensor_scalar(u[:], pa[:], b1col,
                                                        0.0, Add, Max)
                            else:
                                nc.scalar.activation(u[:], pa[:], Relu,
                                                     bias=b1col)
                            # v = max(Pb + b, u): column pool on DVE
                            v = sb.tile([128, N], F32, tag=f"v1_{j2}", bufs=2)
                            nc.vector.scalar_tensor_tensor(
                                v[:], pb[:], b1col, u[:], Add, Max)
                            V.append(v)
                            ei += 1
                        # row pool (relu already folded into u): equal-base
                        nc.vector.tensor_max(r2[t, h][:], V[0][:], V[1][:])

            class Conv2State:
                """conv2 + relu + fc1 accumulation for one N-tile, emitted
                one (yo, s) group at a time so it can interleave with the
                next tile's conv1."""
                def __init__(self, nt, r2):
                    self.nt, self.r2 = nt, r2
                    self.pfc1 = ps.tile([50, N], F32, tag="pfc1", bufs=1,
                                        name=f"pfc1_{nt}")
                    self.pending = []
                    self.gi = 0
                    self.ei = 0

                def group(self):
                    yo, s = divmod(self.ei, 2)
                    nt = self.nt
                    p2 = ps.tile([100, N], F32, tag="p2", bufs=2,
                                 name=f"p2_{nt}_{yo}_{s}")
                    mi = 0
                    for pi in range(2):
                        for h in range(2):
                            co = C_LHST2[yo % 2][pi][s][h]
                            nc.tensor.matmul(
                                p2[:], wr(0, 128, co, co + 100),
                                self.r2[yo // 2 + pi, h][:],
                                start=(mi == 0), stop=(mi == 3))
                            mi += 1
                    a2 = sb.tile([100, N], F32R, tag="a2",
                                 bufs=cfg["a2_bufs"],
                                 name=f"a2_{nt}_{yo}_{s}")
                    if self.ei % cfg["a2_dve_mod"] == cfg["a2_dve_mod"] - 1:
                        nc.vector.tensor_scalar(a2[:], p2[:], b2col[s],
                                                0.0, Add, Max)
                    else:
                        nc.scalar.activation(a2[:], p2[:], Relu,
                                             bias=b2col[s])
                    self.ei += 1
                    self.pending.append((a2, C_LHSTF1[yo][s]))
                    if len(self.pending) > cfg["pending_lag"]:
                        pa2, pcf = self.pending.pop(0)
                        nc.tensor.matmul(
                            self.pfc1[:], wr(0, 100, pcf, pcf + 50),
                            pa2[:], start=(self.gi == 0), stop=False)
                        self.gi += 1

                def flush(self):
                    while self.pending:
                        pa2, pcf = self.pending.pop(0)
                        nc.tensor.matmul(
                            self.pfc1[:], wr(0, 100, pcf, pcf + 50),
                            pa2[:], start=(self.gi == 0),
                            stop=(len(self.pending) == 0))
                        self.gi += 1
                    return self.pfc1

            def tail_stage(nt, pfc1):
                """fc2 + log_softmax + output DMA."""
                n0 = nt * N
                a3 = sb.tile([50, N], F32R, tag="a3", bufs=2,
                             name=f"a3_{nt}")
                nc.scalar.activation(a3[:], pfc1[:], Relu, bias=bf1col)
                zps = ps.tile([10, N], F32, tag="ptail", bufs=1,
                              name=f"zps_{nt}")
                nc.tensor.matmul(zps[:], wr(0, 50, C_LHSTF2, C_LHSTF2 + 10),
                                 a3[:], start=True, stop=True)
                z = sb.tile([10, N], F32, tag="z", bufs=2, name=f"z_{nt}")
                nc.vector.tensor_scalar(z[:], zps[:], Etot, bf2col,
                                        Mult, Add)
                ez = sb.tile([10, N], F32R, tag="ez", bufs=2,
                             name=f"ez_{nt}")
                nc.scalar.activation(ez[:], zps[:], Exp, bias=bf2col,
                                     scale=Etot)
                sps = ps.tile([1, N], F32, tag="ptail", bufs=1,
                               name=f"sps_{nt}")
                nc.tensor.matmul(sps[:], wr(0, 10, C_ONES_COL, C_ONES_COL + 1),
                                 ez[:], start=True, stop=True)
                lse = sb.tile([1, N], F32R, tag="lse", bufs=2,
                              name=f"lse_{nt}")
                nc.scalar.activation(lse[:], sps[:], Ln)
                bps = ps.tile([10, N], F32, tag="ptail", bufs=1,
                               name=f"bps_{nt}")
                nc.tensor.matmul(bps[:], wr(0, 1, C_ONES_ROW, C_ONES_ROW + 10),
                                 lse[:], start=True, stop=True)
                osb = sb.tile([10, N], F32, tag="osb", bufs=2, name=f"osb_{nt}")
                nc.vector.tensor_sub(osb[:], z[:], bps[:])
                nc.sync.dma_start(out[:, n0:n0 + N], osb[:])

            # explicit fine-grained interleave: each conv1 group of tile
            # nt is followed by ~2 conv2 groups of tile nt-1, so the PE has
            # dense matmul work while the pool chains drain on ACT/DVE
            for _rep in range(repeat):
                prev = None  # Conv2State of tile nt-1
                for nt in range(n_tiles):
                    r2 = alloc_r2(nt)
                    ci = 0
                    for g in range(12):
                        t, h = divmod(g, 2)
                        conv1_group(nt, t, h, r2, g)
                        if prev is not None:
                            want = 20 * (g + 1) // 12
                            while ci < want:
                                prev.group()
                                ci += 1
                    if prev is not None:
                        tail_stage(prev.nt, prev.flush())
                    prev = Conv2State(nt, r2)
                while prev.ei < 20:
                    prev.group()
                tail_stage(prev.nt, prev.flush())
    nc.compile()
    return nc


def kernel(**inputs):
    global LAST_EXEC_TIME_NS, LAST_RESULTS
    x = np.ascontiguousarray(np.asarray(inputs["x"], dtype=np.float32))
    wb16, wb, scales = _host_prep({k: np.asarray(v) for k, v in inputs.items()})

    nc = build_program(scales)

    import ml_dtypes
    wb16_b = wb16.astype(ml_dtypes.bfloat16)
    in_maps = []
    for i in range(N_CORES):
        xs = x[i * BC:(i + 1) * BC, 0]            # [BC, 28, 28]
        xTi = np.ascontiguousarray(
            xs.transpose(1, 2, 0).astype(ml_dtypes.bfloat16))  # [28, 28, BC]
        in_maps.append({"xT": xTi, "wblob16": wb16_b, "wblob": wb})

    trace = bool(os.environ.get("KERNEL_TRACE"))
    res = run_bass_kernel_spmd(nc, in_maps, list(range(N_CORES)), trace=trace)
    LAST_EXEC_TIME_NS = res.exec_time_ns
    LAST_RESULTS = res

    out = np.empty((B_TOTAL, 10), np.float32)
    for i in range(N_CORES):
        out[i * BC:(i + 1) * BC] = res.results[i]["out"].T
    return out


# revision 10
# speedup vs baseline: 1.1426x; 1.0072x over previous
"""Trainium2 Bass kernel for the binarized ConvNet (nn_ConvNet_81501299409071).

Data-parallel over batch: 8192 images -> 8 NeuronCores x 1024 images.

Device pipeline (feature-major: features on partitions, batch on free dim),
everything is a matmul against exactly-representable +-1 Toeplitz weight
matrices; the DoReFa binarization scale E is folded into the post-matmul
activation ops (relu(acc*E + b)).

  conv1 5x5 (1->10ch):  bf16 x, 6x2 input tiles [128=(8 rows x 16 cols),
      N=512], 4 matmuls of M=120 per tile; M packed as (row-in-pair, ch,
      col-pair) so the 2x2 maxpool+bias+relu is a 4-deep chain
      u=relu(pa0+b) (ACT) then three scalar_tensor_tensor folds (DVE).
  conv2 3x3 (10->20ch): per output row, 3 accumulating K=120 matmuls x 2
      output-channel halves (fp32r).
  fc1 2000->50: 20 accumulating K=100 matmuls (one per conv2 relu tile).
  fc2 50->10 + log_softmax: exp/ln on ACT, partition sum / broadcast via
      tiny ones-matmuls, final subtract on DVE.

Startup: conv1 weights (bf16) + biases land via small leading DMAs on the
gpsimd queue; rhs DMAs stream on sync; a short burst of dummy matmuls on a
zeroed scratch tile warms the PE HAM clock-gate during the initial DMA wait.
"""
import os
import numpy as np

import concourse.bass as bass
import concourse.tile as tile
from concourse import bacc, mybir
from concourse.bass_utils import run_bass_kernel_spmd

F32 = mybir.dt.float32
F32R = mybir.dt.float32r
BF16 = mybir.dt.bfloat16

N_CORES = 8
B_TOTAL = 8192
BC = B_TOTAL // N_CORES  # 1024 images per core
N = 512                  # batch tile (free dim / PSUM bank)
N_TILES = BC // N

LAST_EXEC_TIME_NS = None
LAST_RESULTS = None

# ---------------------------------------------------------------------------
# bf16 weight blob: conv1 Toeplitz blocks (loads first, tiny)
# ---------------------------------------------------------------------------
C16_LHST1 = [[128 * (2 * j2 + par) for par in range(2)] for j2 in range(2)]
WB16_COLS = 512

# ---------------------------------------------------------------------------
# f32 weight blob: biases first (needed by the first pool ops), then conv2,
# fc1, fc2, ones — ordered so DMA chunks arrive in the order the pipeline
# needs them.
# ---------------------------------------------------------------------------
_off = 0
def _take(n):
    global _off
    c = _off
    _off += n
    return c

C_B1 = _take(1)               # [128,1]
C_B2 = [_take(1) for _s in range(2)]  # [100,1] each
CHUNK_A = _off                # biases chunk (tiny, first)
C_LHST2 = [[[[_take(100) for _h in range(2)] for _s in range(2)]
            for _pi in range(2)] for _pyo in range(2)]  # [yo%2][pair][s][h]
CHUNK_B = _off                # + conv2 weights
C_LHSTF1 = [[_take(50) for _s in range(2)] for _yo in range(10)]    # [yo][s]
C_LHSTF2 = _take(10)          # K=50
C_ONES_ROW = _take(10)        # [1,10] ones (broadcast lhsT)
C_ONES_COL = _take(1)         # [10,1] ones (partition-sum lhsT)
C_BF1 = _take(1)              # [50,1]
C_BF2 = _take(1)              # [10,1]
WBLOB_COLS = _off


def _host_prep(inputs):
    """Binarize weights, build +-1 Toeplitz matrices + bias columns packed
    into the blobs, and the E scales."""
    w1, b1 = inputs["w1"], inputs["b1"]
    w2, b2 = inputs["w2"], inputs["b2"]
    fw1, fb1 = inputs["fw1"], inputs["fb1"]
    fw2, fb2 = inputs["fw2"], inputs["fb2"]

    scales = {
        "E1": float(np.mean(np.abs(w1))),
        "E2": float(np.mean(np.abs(w2))),
        "Ef1": float(np.mean(np.abs(fw1))),
        "Ef2": float(np.mean(np.abs(fw2))),
    }
    s1 = np.sign(w1).astype(np.float32)
    s2 = np.sign(w2).astype(np.float32)
    sf1 = np.sign(fw1).astype(np.float32)
    sf2 = np.sign(fw2).astype(np.float32)

    # bf16 conv1 blob: [j2][par], K p = r*16 + xi, M m = jp*64 + oc*6 + c
    wb16 = np.zeros((128, WB16_COLS), np.float32)
    for j2 in range(2):
        for par in range(2):
            co = C16_LHST1[j2][par]
            for jp in range(2):
                j = 2 * jp + j2
                for oc in range(10):
                    for c in range(6):
                        m = jp * 64 + oc * 6 + c
                        xo = 2 * c + par
                        for dy in range(5):
                            for dx in range(5):
                                wb16[(j + dy) * 16 + xo + dx, co + m] = \
                                    s1[oc, 0, dy, dx]

    wb = np.zeros((128, WBLOB_COLS), np.float32)

    # conv2 Toeplitz [yo%2][pair pi][s][h]: K p = rp*64 + ci*6 + c where rp
    # is row-in-pair of pooled pair floor(yo/2)+pi; M m = oci*10 + xo
    for pyo in range(2):
        for pi in range(2):
            for s_ in range(2):
                for h in range(2):
                    blk = np.zeros((128, 100), np.float32)
                    for rp in range(2):
                        dy = 2 * pi + rp - pyo
                        if not (0 <= dy <= 2):
                            continue
                        for ci in range(10):
                            for c in range(6):
                                pp = rp * 64 + ci * 6 + c
                                xi = 6 * h + c
                                for oci in range(10):
                                    for xo in range(10):
                                        dx = xi - xo
                                        if 0 <= dx < 3:
                                            blk[pp, oci * 10 + xo] = \
                                                s2[10 * s_ + oci, ci, dy, dx]
                    co = C_LHST2[pyo][pi][s_][h]
                    wb[:, co:co + 100] = blk

    # fc1 [yo][s]: K p = oci*10+xo -> f = (10s+oci)*100 + yo*10 + xo
    for yo in range(10):
        for s in range(2):
            blk = np.zeros((100, 50), np.float32)
            for oci in range(10):
                for xo in range(10):
                    f = (10 * s + oci) * 100 + yo * 10 + xo
                    blk[oci * 10 + xo, :] = sf1[:, f]
            co = C_LHSTF1[yo][s]
            wb[0:100, co:co + 50] = blk

    wb[0:50, C_LHSTF2:C_LHSTF2 + 10] = sf2.T
    wb[0, C_ONES_ROW:C_ONES_ROW + 10] = 1.0
    wb[0:10, C_ONES_COL] = 1.0

    # bias columns, pre-divided by the accumulated binarization scales so
    # every bias+relu runs unscaled (relu(acc + b')) on any engine; the one
    # true scale Etot is applied at the logits.
    E1, E2, Ef1 = scales["E1"], scales["E2"], scales["Ef1"]
    b1v = np.zeros(128, np.float32)
    for jp in range(2):
        for ci in range(10):
            b1v[jp * 64 + ci * 6:jp * 64 + ci * 6 + 6] = b1[ci] / E1
    wb[:, C_B1] = b1v
    for s in range(2):
        b2v = np.repeat(b2[10 * s:10 * s + 10], 10).astype(np.float32)
        wb[0:100, C_B2[s]] = b2v / (E1 * E2)
    wb[0:50, C_BF1] = fb1 / (E1 * E2 * Ef1)
    wb[0:10, C_BF2] = fb2
    return wb16, wb, scales


# tuning knobs (engine splits / pool sizing), overridable for sweeps
CFG = {
    "n_warm": 12,         # dummy warmup MMs to heat the PE clock gate
    "rhs_bufs": 12,
    "p1_bufs": 2,
    "a2_bufs": 4,
    "pending_lag": 2,     # fc1 matmul lag behind a2 relu
    "a2_dve_mod": 1000,   # a2 relu -> DVE when idx % mod == mod-1
    "u_dve_mod": 0,       # u op -> DVE when idx % mod == mod-1 (0 = never)
    "tail_halves": 2,     # split the softmax tail into this many chunks
}


def build_program(scales, n_tiles=N_TILES, bc=BC, cfg=None, repeat=1):
    """Build the single-core SPMD bass program."""
    cfg = {**CFG, **(cfg or {})}
    Etot = scales["E1"] * scales["E2"] * scales["Ef1"] * scales["Ef2"]
    Relu = mybir.ActivationFunctionType.Relu
    Exp = mybir.ActivationFunctionType.Exp
    Ln = mybir.ActivationFunctionType.Ln
    Add = mybir.AluOpType.add
    Max = mybir.AluOpType.max
    Mult = mybir.AluOpType.mult

    nc = bacc.Bacc("TRN2", target_bir_lowering=False, debug=False)
    xT = nc.dram_tensor("xT", [28, 28, bc], BF16, kind="ExternalInput").ap()
    wblob16 = nc.dram_tensor("wblob16", [128, WB16_COLS], BF16,
                             kind="ExternalInput").ap()
    # declared float32r so fp32r matmuls may consume them directly (walrus
    # requires fp32r operands to come from fp32r-emitting producers); host
    # supplies plain fp32 bits
    wblob = nc.dram_tensor("wblob", [128, WBLOB_COLS], F32R,
                           kind="ExternalInput").ap()
    out = nc.dram_tensor("out", [10, bc], F32, kind="ExternalOutput").ap()

    with tile.TileContext(nc) as tc:
        with tc.tile_pool(name="wpool", bufs=1) as wpool, \
             tc.tile_pool(name="sb", bufs=1) as sb, \
             tc.tile_pool(name="ps", bufs=1, space="PSUM") as ps:

            wb16 = wpool.tile([128, WB16_COLS], BF16, tag="wb16")
            wb = wpool.tile([128, WBLOB_COLS], F32R, tag="wb")
            # weight blobs on the gpsimd queue: conv1(bf16) + biases first
            # (small), then conv2, then the fc tail weights
            nc.gpsimd.dma_start(wb16[:], wblob16[:])
            nc.gpsimd.dma_start(wb[:, 0:CHUNK_A], wblob[:, 0:CHUNK_A])
            nc.gpsimd.dma_start(wb[:, CHUNK_A:CHUNK_B],
                                wblob[:, CHUNK_A:CHUNK_B])
            nc.gpsimd.dma_start(wb[:, CHUNK_B:WBLOB_COLS],
                                wblob[:, CHUNK_B:WBLOB_COLS])

            # HAM warmup: dummy matmuls on a zeroed scratch tile keep the PE
            # busy while the first real rhs tiles stream in
            if cfg["n_warm"]:
                warm = sb.tile([128, N], BF16, tag="warm")
                nc.vector.memzero(warm[:])
                wps = ps.tile([128, N], F32, tag="ptail", bufs=1,
                              name="warm_ps")
                for wi in range(cfg["n_warm"]):
                    nc.tensor.matmul(wps[:], warm[:, 0:128], warm[:],
                                     start=True, stop=True)

            def wr(p0, p1, c0, c1):  # f32r slice of the weight blob
                return wb[p0:p1, c0:c1]

            b1col = wb[0:128, C_B1:C_B1 + 1].bitcast(F32)
            b2col = [wb[0:100, C_B2[s]:C_B2[s] + 1].bitcast(F32)
                     for s in range(2)]
            bf1col = wb[0:50, C_BF1:C_BF1 + 1].bitcast(F32)
            bf2col = wb[0:10, C_BF2:C_BF2 + 1].bitcast(F32)

            # per-N-tile stage emitters -----------------------------------
            def alloc_r2(nt):
                r2 = {}
                for q in range(6):
                    for hh in range(2):
                        r2[q, hh] = sb.tile([128, N], F32R,
                                            tag=f"r2_{q}_{hh}", bufs=2,
                                            name=f"r2_{q}_{hh}_{nt}")
                return r2

            def conv1_group(nt, t, h, r2, ei):
                """One conv1 rhs tile: DMA + 4 matmuls + 2x2 maxpool chain
                (bias+relu fused) into r2[(t, h)]."""
                n0 = nt * N
                if True:
                    if True:
                        rhs = sb.tile([128, N], BF16, tag="rhs1",
                                      bufs=cfg["rhs_bufs"])
                        nc.sync.dma_start(
                            rhs[:], xT[4 * t:4 * t + 8, 12 * h:12 * h + 16,
                                       n0:n0 + N])
                        V = []
                        for j2 in range(2):
                            pa = ps.tile([128, N], F32, tag="p1e",
                                         bufs=cfg["p1_bufs"],
                                         name=f"p1e_{nt}_{t}_{h}_{j2}")
                            pb = ps.tile([128, N], F32, tag="p1o",
                                         bufs=cfg["p1_bufs"],
                                         name=f"p1o_{nt}_{t}_{h}_{j2}")
                            for par, p_ in ((0, pa), (1, pb)):
                                co = C16_LHST1[j2][par]
                                nc.tensor.matmul(p_[:],
                                                 wb16[:, co:co + 128],
                                                 rhs[:], start=True, stop=True)
                            # u = relu(Pa + b): the relu makes the final
                            # plain max-fold equal relu(pool+b)
                            u = sb.tile([128, N], F32, tag="u1", bufs=3)
                            if cfg["u_dve_mod"] and \
                                    ei % cfg["u_dve_mod"] == \
                                    cfg["u_dve_mod"] - 1:
                                nc.vector.tensor_scalar(u[:], pa[:], b1col,
                                                        0.0, Add, Max)
                            else:
                                nc.scalar.activation(u[:], pa[:], Relu,
                                                     bias=b1col)
                            # v = max(Pb + b, u): column pool on DVE
                            v = sb.tile([128, N], F32, tag=f"v1_{j2}", bufs=2)
                            nc.vector.scalar_tensor_tensor(
                                v[:], pb[:], b1col, u[:], Add, Max)
                            V.append(v)
                            ei += 1
                        # row pool (relu already folded into u): equal-base
                        nc.vector.tensor_max(r2[t, h][:], V[0][:], V[1][:])

            class Conv2State:
                """conv2 + relu + fc1 accumulation for one N-tile, emitted
                one (yo, s) group at a time so it can interleave with the
                next tile's conv1."""
                def __init__(self, nt, r2):
                    self.nt, self.r2 = nt, r2
                    self.pfc1 = ps.tile([50, N], F32, tag="pfc1", bufs=1,
                                        name=f"pfc1_{nt}")
                    self.pending = []
                    self.gi = 0
                    self.ei = 0

                def group(self):
                    yo, s = divmod(self.ei, 2)
                    nt = self.nt
                    p2 = ps.tile([100, N], F32, tag="p2", bufs=2,
                                 name=f"p2_{nt}_{yo}_{s}")
                    mi = 0
                    for pi in range(2):
                        for h in range(2):
                            co = C_LHST2[yo % 2][pi][s][h]
                            nc.tensor.matmul(
                                p2[:], wr(0, 128, co, co + 100),
                                self.r2[yo // 2 + pi, h][:],
                                start=(mi == 0), stop=(mi == 3))
                            mi += 1
                    a2 = sb.tile([100, N], F32R, tag="a2",
                                 bufs=cfg["a2_bufs"],
                                 name=f"a2_{nt}_{yo}_{s}")
                    if self.ei % cfg["a2_dve_mod"] == cfg["a2_dve_mod"] - 1:
                        nc.vector.tensor_scalar(a2[:], p2[:], b2col[s],
                                                0.0, Add, Max)
                    else:
                        nc.scalar.activation(a2[:], p2[:], Relu,
                                             bias=b2col[s])
                    self.ei += 1
                    self.pending.append((a2, C_LHSTF1[yo][s]))
                    if len(self.pending) > cfg["pending_lag"]:
                        pa2, pcf = self.pending.pop(0)
                        nc.tensor.matmul(
                            self.pfc1[:], wr(0, 100, pcf, pcf + 50),
                            pa2[:], start=(self.gi == 0), stop=False)
                        self.gi += 1

                def flush(self):
                    while self.pending:
                        pa2, pcf = self.pending.pop(0)
                        nc.tensor.matmul(
                            self.pfc1[:], wr(0, 100, pcf, pcf + 50),
                            pa2[:], start=(self.gi == 0),
                            stop=(len(self.pending) == 0))
                        self.gi += 1
                    return self.pfc1

            def tail_stage(nt, pfc1):
                """fc2 + log_softmax + output DMA, pipelined in chunks so
                the serial exp/ln chain isn't fully exposed."""
                nh = cfg["tail_halves"]
                NH = N // nh
                a3 = sb.tile([50, N], F32R, tag="a3", bufs=2,
                             name=f"a3_{nt}")
                for hx in range(nh):
                    c0, c1 = hx * NH, (hx + 1) * NH
                    n0 = nt * N + c0
                    nc.scalar.activation(a3[:, c0:c1], pfc1[:, c0:c1], Relu,
                                         bias=bf1col)
                    ptag = "ptail" if hx % 2 == 0 else "pfc1"
                    zps = ps.tile([10, NH], F32, tag=ptag, bufs=1,
                                  name=f"zps_{nt}_{hx}")
                    nc.tensor.matmul(zps[:],
                                     wr(0, 50, C_LHSTF2, C_LHSTF2 + 10),
                                     a3[:, c0:c1], start=True, stop=True)
                    z = sb.tile([10, NH], F32, tag="z", bufs=2,
                                name=f"z_{nt}_{hx}")
                    nc.vector.tensor_scalar(z[:], zps[:], Etot, bf2col,
                                            Mult, Add)
                    ez = sb.tile([10, NH], F32R, tag="ez", bufs=2,
                                 name=f"ez_{nt}_{hx}")
                    nc.scalar.activation(ez[:], zps[:], Exp, bias=bf2col,
                                         scale=Etot)
                    sps = ps.tile([1, NH], F32, tag=ptag, bufs=1,
                                  name=f"sps_{nt}_{hx}")
                    nc.tensor.matmul(sps[:],
                                     wr(0, 10, C_ONES_COL, C_ONES_COL + 1),
                                     ez[:], start=True, stop=True)
                    lse = sb.tile([1, NH], F32R, tag="lse", bufs=2,
                                  name=f"lse_{nt}_{hx}")
                    nc.scalar.activation(lse[:], sps[:], Ln)
                    bps = ps.tile([10, NH], F32, tag=ptag, bufs=1,
                                  name=f"bps_{nt}_{hx}")
                    nc.tensor.matmul(bps[:],
                                     wr(0, 1, C_ONES_ROW, C_ONES_ROW + 10),
                                     lse[:], start=True, stop=True)
                    osb = sb.tile([10, NH], F32, tag="osb", bufs=2,
                                  name=f"osb_{nt}_{hx}")
                    nc.vector.tensor_sub(osb[:], z[:], bps[:])
                    nc.sync.dma_start(out[:, n0:n0 + NH], osb[:])

            # explicit fine-grained interleave: each conv1 group of tile
            # nt is followed by ~2 conv2 groups of tile nt-1, so the PE has
            # dense matmul work while the pool chains drain on ACT/DVE
            for _rep in range(repeat):
                prev = None  # Conv2State of tile nt-1
                for nt in range(n_tiles):
                    r2 = alloc_r2(nt)
                    ci = 0
                    for g in range(12):
                        t, h = divmod(g, 2)
                        conv1_group(nt, t, h, r2, g)
                        if prev is not None:
                            want = 20 * (g + 1) // 12
                            while ci < want:
                                prev.group()
                                ci += 1
                    if prev is not None:
                        tail_stage(prev.nt, prev.flush())
                    prev = Conv2State(nt, r2)
                while prev.ei < 20:
                    prev.group()
                tail_stage(prev.nt, prev.flush())
    nc.compile()
    return nc


def kernel(**inputs):
    global LAST_EXEC_TIME_NS, LAST_RESULTS
    x = np.ascontiguousarray(np.asarray(inputs["x"], dtype=np.float32))
    wb16, wb, scales = _host_prep({k: np.asarray(v) for k, v in inputs.items()})

    nc = build_program(scales)

    import ml_dtypes
    wb16_b = wb16.astype(ml_dtypes.bfloat16)
    in_maps = []
    for i in range(N_CORES):
        xs = x[i * BC:(i + 1) * BC, 0]            # [BC, 28, 28]
        xTi = np.ascontiguousarray(
            xs.transpose(1, 2, 0).astype(ml_dtypes.bfloat16))  # [28, 28, BC]
        in_maps.append({"xT": xTi, "wblob16": wb16_b, "wblob": wb})

    trace = bool(os.environ.get("KERNEL_TRACE"))
    res = run_bass_kernel_spmd(nc, in_maps, list(range(N_CORES)), trace=trace)
    LAST_EXEC_TIME_NS = res.exec_time_ns
    LAST_RESULTS = res

    out = np.empty((B_TOTAL, 10), np.float32)
    for i in range(N_CORES):
        out[i * BC:(i + 1) * BC] = res.results[i]["out"].T
    return out


# revision 15
# speedup vs baseline: 1.2139x; 1.0624x over previous
"""Trainium2 Bass kernel for the binarized ConvNet (nn_ConvNet_81501299409071).

Data-parallel over batch: 8192 images -> 8 NeuronCores x 1024 images.

Device pipeline (feature-major: features on partitions, batch on free dim),
everything is a matmul against exactly-representable +-1 Toeplitz weight
matrices; the DoReFa binarization scale E is folded into the post-matmul
activation ops (relu(acc*E + b)).

  conv1 5x5 (1->10ch):  bf16 x, 6x2 input tiles [128=(8 rows x 16 cols),
      N=512], 4 matmuls of M=120 per tile; M packed as (row-in-pair, ch,
      col-pair) so the 2x2 maxpool+bias+relu is a 4-deep chain
      u=relu(pa0+b) (ACT) then three scalar_tensor_tensor folds (DVE).
  conv2 3x3 (10->20ch): per output row, 3 accumulating K=120 matmuls x 2
      output-channel halves (fp32r).
  fc1 2000->50: 20 accumulating K=100 matmuls (one per conv2 relu tile).
  fc2 50->10 + log_softmax: exp/ln on ACT, partition sum / broadcast via
      tiny ones-matmuls, final subtract on DVE.

Startup: conv1 weights (bf16) + biases land via small leading DMAs on the
gpsimd queue; rhs DMAs stream on sync; a short burst of dummy matmuls on a
zeroed scratch tile warms the PE HAM clock-gate during the initial DMA wait.
"""
import os
import numpy as np

import concourse.bass as bass
import concourse.tile as tile
from concourse import bacc, mybir
from concourse.bass_utils import run_bass_kernel_spmd

F32 = mybir.dt.float32
F32R = mybir.dt.float32r
BF16 = mybir.dt.bfloat16

N_CORES = 8
B_TOTAL = 8192
BC = B_TOTAL // N_CORES  # 1024 images per core
N = 512                  # batch tile (free dim / PSUM bank)
N_TILES = BC // N

LAST_EXEC_TIME_NS = None
LAST_RESULTS = None

# ---------------------------------------------------------------------------
# bf16 weight blob: conv1 Toeplitz blocks (loads first, tiny)
# ---------------------------------------------------------------------------
C16_LHST1 = [[128 * (2 * j2 + par) for par in range(2)] for j2 in range(2)]
WB16_COLS = 512

# ---------------------------------------------------------------------------
# f32 weight blob: biases first (needed by the first pool ops), then conv2,
# fc1, fc2, ones — ordered so DMA chunks arrive in the order the pipeline
# needs them.
# ---------------------------------------------------------------------------
_off = 0
def _take(n):
    global _off
    c = _off
    _off += n
    return c

C_B1 = _take(1)               # [128,1]
C_B2 = [_take(1) for _s in range(2)]  # [100,1] each
CHUNK_A = _off                # biases chunk (tiny, first)
C_LHST2 = [[[[_take(100) for _h in range(2)] for _s in range(2)]
            for _pi in range(2)] for _pyo in range(2)]  # [yo%2][pair][s][h]
CHUNK_B = _off                # + conv2 weights
C_LHSTF2 = _take(10)          # K=50
C_ONES_ROW = _take(10)        # [1,10] ones (broadcast lhsT)
C_ONES_COL = _take(1)         # [10,1] ones (partition-sum lhsT)
C_BF1 = _take(1)              # [50,1]
C_BF2 = _take(1)              # [10,1]
WBLOB_COLS = _off

# fp8 DoubleRow fc1 blob: per yo a [100, 2, 64] block (pair dim = s, M
# padded 50->64 for the stride-16 LDWEIGHTS rule), laid out in one
# [128, 10*128] fp8 tensor
W8_YO = 128
WBLOB8_COLS = 10 * W8_YO


def _host_prep(inputs):
    """Binarize weights, build +-1 Toeplitz matrices + bias columns packed
    into the blobs, and the E scales."""
    w1, b1 = inputs["w1"], inputs["b1"]
    w2, b2 = inputs["w2"], inputs["b2"]
    fw1, fb1 = inputs["fw1"], inputs["fb1"]
    fw2, fb2 = inputs["fw2"], inputs["fb2"]

    scales = {
        "E1": float(np.mean(np.abs(w1))),
        "E2": float(np.mean(np.abs(w2))),
        "Ef1": float(np.mean(np.abs(fw1))),
        "Ef2": float(np.mean(np.abs(fw2))),
    }
    s1 = np.sign(w1).astype(np.float32)
    s2 = np.sign(w2).astype(np.float32)
    sf1 = np.sign(fw1).astype(np.float32)
    sf2 = np.sign(fw2).astype(np.float32)

    # bf16 conv1 blob: [j2][par], K p = r*16 + xi, M m = jp*64 + oc*6 + c
    wb16 = np.zeros((128, WB16_COLS), np.float32)
    for j2 in range(2):
        for par in range(2):
            co = C16_LHST1[j2][par]
            for jp in range(2):
                j = 2 * jp + j2
                for oc in range(10):
                    for c in range(6):
                        m = jp * 64 + oc * 6 + c
                        xo = 2 * c + par
                        for dy in range(5):
                            for dx in range(5):
                                wb16[(j + dy) * 16 + xo + dx, co + m] = \
                                    s1[oc, 0, dy, dx]

    wb = np.zeros((128, WBLOB_COLS), np.float32)

    # conv2 Toeplitz [yo%2][pair pi][s][h]: K p = rp*64 + ci*6 + c where rp
    # is row-in-pair of pooled pair floor(yo/2)+pi; M m = oci*10 + xo
    for pyo in range(2):
        for pi in range(2):
            for s_ in range(2):
                for h in range(2):
                    blk = np.zeros((128, 100), np.float32)
                    for rp in range(2):
                        dy = 2 * pi + rp - pyo
                        if not (0 <= dy <= 2):
                            continue
                        for ci in range(10):
                            for c in range(6):
                                pp = rp * 64 + ci * 6 + c
                                xi = 6 * h + c
                                for oci in range(10):
                                    for xo in range(10):
                                        dx = xi - xo
                                        if 0 <= dx < 3:
                                            blk[pp, oci * 10 + xo] = \
                                                s2[10 * s_ + oci, ci, dy, dx]
                    co = C_LHST2[pyo][pi][s_][h]
                    wb[:, co:co + 100] = blk

    # fc1 fp8 DR blob [yo]: K p = oci*10+xo, pair ko = s,
    # f = (10*ko+oci)*100 + yo*10 + xo
    wb8 = np.zeros((128, WBLOB8_COLS), np.float32)
    for yo in range(10):
        for ko in range(2):
            for oci in range(10):
                for xo in range(10):
                    f = (10 * ko + oci) * 100 + yo * 10 + xo
                    wb8[oci * 10 + xo,
                        yo * W8_YO + ko * 64: yo * W8_YO + ko * 64 + 50] = \
                        sf1[:, f]

    wb[0:50, C_LHSTF2:C_LHSTF2 + 10] = sf2.T
    wb[0, C_ONES_ROW:C_ONES_ROW + 10] = 1.0
    wb[0:10, C_ONES_COL] = 1.0

    # bias columns, pre-divided by the accumulated binarization scales so
    # every bias+relu runs unscaled (relu(acc + b')) on any engine; the one
    # true scale Etot is applied at the logits.
    E1, E2, Ef1 = scales["E1"], scales["E2"], scales["Ef1"]
    b1v = np.zeros(128, np.float32)
    for jp in range(2):
        for ci in range(10):
            b1v[jp * 64 + ci * 6:jp * 64 + ci * 6 + 6] = b1[ci] / E1
    wb[:, C_B1] = b1v
    for s in range(2):
        b2v = np.repeat(b2[10 * s:10 * s + 10], 10).astype(np.float32)
        wb[0:100, C_B2[s]] = b2v
    wb[0:50, C_BF1] = fb1 / (E1 * E2 * Ef1)
    wb[0:10, C_BF2] = fb2
    return wb16, wb, wb8, scales


# tuning knobs (engine splits / pool sizing), overridable for sweeps
CFG = {
    "n_warm": 12,         # dummy warmup MMs to heat the PE clock gate
    "rhs_bufs": 12,
    "p1_bufs": 2,
    "a2_bufs": 4,
    "pending_lag": 1,     # fc1 matmul lag (in yo units)
    "a2_dve_mod": 1000,   # a2 relu -> DVE when idx % mod == mod-1
    "u_dve_mod": 0,       # u op -> DVE when idx % mod == mod-1 (0 = never)
    "tail_halves": 2,     # split the softmax tail into this many chunks
}


def build_program(scales, n_tiles=N_TILES, bc=BC, cfg=None, repeat=1):
    """Build the single-core SPMD bass program."""
    cfg = {**CFG, **(cfg or {})}
    Etot = scales["E1"] * scales["E2"] * scales["Ef1"] * scales["Ef2"]
    Relu = mybir.ActivationFunctionType.Relu
    Exp = mybir.ActivationFunctionType.Exp
    Ln = mybir.ActivationFunctionType.Ln
    Add = mybir.AluOpType.add
    Max = mybir.AluOpType.max
    Mult = mybir.AluOpType.mult

    E12 = scales["E1"] * scales["E2"]
    FP8 = mybir.dt.float8e4
    DR = mybir.MatmulPerfMode.DoubleRow
    nc = bacc.Bacc("TRN2", target_bir_lowering=False, debug=False)
    xT = nc.dram_tensor("xT", [28, 28, bc], BF16, kind="ExternalInput").ap()
    wblob16 = nc.dram_tensor("wblob16", [128, WB16_COLS], BF16,
                             kind="ExternalInput").ap()
    # declared float32r so fp32r matmuls may consume them directly (walrus
    # requires fp32r operands to come from fp32r-emitting producers); host
    # supplies plain fp32 bits
    wblob = nc.dram_tensor("wblob", [128, WBLOB_COLS], F32R,
                           kind="ExternalInput").ap()
    wblob8 = nc.dram_tensor("wblob8", [128, WBLOB8_COLS], FP8,
                            kind="ExternalInput").ap()
    out = nc.dram_tensor("out", [10, bc], F32, kind="ExternalOutput").ap()

    with tile.TileContext(nc) as tc:
        with tc.tile_pool(name="wpool", bufs=1) as wpool, \
             tc.tile_pool(name="sb", bufs=1) as sb, \
             tc.tile_pool(name="ps", bufs=1, space="PSUM") as ps:

            wb16 = wpool.tile([128, WB16_COLS], BF16, tag="wb16")
            wb = wpool.tile([128, WBLOB_COLS], F32R, tag="wb")
            wb8 = wpool.tile([128, WBLOB8_COLS], FP8, tag="wb8")
            # weight blobs on the gpsimd queue: conv1(bf16) + biases first
            # (small), then conv2, then the fc tail weights
            nc.gpsimd.dma_start(wb16[:], wblob16[:])
            nc.gpsimd.dma_start(wb[:, 0:CHUNK_A], wblob[:, 0:CHUNK_A])
            nc.gpsimd.dma_start(wb[:, CHUNK_A:CHUNK_B],
                                wblob[:, CHUNK_A:CHUNK_B])
            nc.gpsimd.dma_start(wb[:, CHUNK_B:WBLOB_COLS],
                                wblob[:, CHUNK_B:WBLOB_COLS])
            nc.gpsimd.dma_start(wb8[:], wblob8[:])

            # HAM warmup: dummy matmuls on a zeroed scratch tile keep the PE
            # busy while the first real rhs tiles stream in
            if cfg["n_warm"]:
                warm = sb.tile([128, N], BF16, tag="warm")
                nc.vector.memzero(warm[:])
                wps = ps.tile([128, N], F32, tag="ptail", bufs=1,
                              name="warm_ps")
                for wi in range(cfg["n_warm"]):
                    nc.tensor.matmul(wps[:], warm[:, 0:128], warm[:],
                                     start=True, stop=True)

            def wr(p0, p1, c0, c1):  # f32r slice of the weight blob
                return wb[p0:p1, c0:c1]

            b1col = wb[0:128, C_B1:C_B1 + 1].bitcast(F32)
            b2col = [wb[0:100, C_B2[s]:C_B2[s] + 1].bitcast(F32)
                     for s in range(2)]
            bf1col = wb[0:50, C_BF1:C_BF1 + 1].bitcast(F32)
            bf2col = wb[0:10, C_BF2:C_BF2 + 1].bitcast(F32)

            # per-N-tile stage emitters -----------------------------------
            def alloc_r2(nt):
                r2 = {}
                for q in range(6):
                    for hh in range(2):
                        r2[q, hh] = sb.tile([128, N], F32R,
                                            tag=f"r2_{q}_{hh}", bufs=2,
                                            name=f"r2_{q}_{hh}_{nt}")
                return r2

            def conv1_group(nt, t, h, r2, ei):
                """One conv1 rhs tile: DMA + 4 matmuls + 2x2 maxpool chain
                (bias+relu fused) into r2[(t, h)]."""
                n0 = nt * N
                if True:
                    if True:
                        rhs = sb.tile([128, N], BF16, tag="rhs1",
                                      bufs=cfg["rhs_bufs"])
                        nc.sync.dma_start(
                            rhs[:], xT[4 * t:4 * t + 8, 12 * h:12 * h + 16,
                                       n0:n0 + N])
                        V = []
                        for j2 in range(2):
                            pa = ps.tile([128, N], F32, tag="p1e",
                                         bufs=cfg["p1_bufs"],
                                         name=f"p1e_{nt}_{t}_{h}_{j2}")
                            pb = ps.tile([128, N], F32, tag="p1o",
                                         bufs=cfg["p1_bufs"],
                                         name=f"p1o_{nt}_{t}_{h}_{j2}")
                            for par, p_ in ((0, pa), (1, pb)):
                                co = C16_LHST1[j2][par]
                                nc.tensor.matmul(p_[:],
                                                 wb16[:, co:co + 128],
                                                 rhs[:], start=True, stop=True)
                            # u = relu(Pa + b): the relu makes the final
                            # plain max-fold equal relu(pool+b)
                            u = sb.tile([128, N], F32, tag="u1", bufs=3)
                            if cfg["u_dve_mod"] and \
                                    ei % cfg["u_dve_mod"] == \
                                    cfg["u_dve_mod"] - 1:
                                nc.vector.tensor_scalar(u[:], pa[:], b1col,
                                                        0.0, Add, Max)
                            else:
                                nc.scalar.activation(u[:], pa[:], Relu,
                                                     bias=b1col)
                            # v = max(Pb + b, u): column pool on DVE
                            v = sb.tile([128, N], F32, tag=f"v1_{j2}", bufs=2)
                            nc.vector.scalar_tensor_tensor(
                                v[:], pb[:], b1col, u[:], Add, Max)
                            V.append(v)
                            ei += 1
                        # row pool (relu already folded into u): equal-base
                        nc.vector.tensor_max(r2[t, h][:], V[0][:], V[1][:])

            class Conv2State:
                """conv2 + relu + fc1 accumulation for one N-tile, emitted
                one (yo, s) group at a time so it can interleave with the
                next tile's conv1. a2 is stored fp8 at natural scale; fc1
                runs as 10 DoubleRow K=200 matmuls (pair dim = s)."""
                def __init__(self, nt, r2):
                    self.nt, self.r2 = nt, r2
                    self.pfc1 = ps.tile([64, N], F32, tag="pfc1", bufs=1,
                                        name=f"pfc1_{nt}")
                    self.pending = []
                    self.gi = 0
                    self.ei = 0
                    self.a2cur = None

                def group(self):
                    yo, s = divmod(self.ei, 2)
                    nt = self.nt
                    p2 = ps.tile([100, N], F32, tag="p2", bufs=2,
                                 name=f"p2_{nt}_{yo}_{s}")
                    mi = 0
                    for pi in range(2):
                        for h in range(2):
                            co = C_LHST2[yo % 2][pi][s][h]
                            nc.tensor.matmul(
                                p2[:], wr(0, 128, co, co + 100),
                                self.r2[yo // 2 + pi, h][:],
                                start=(mi == 0), stop=(mi == 3))
                            mi += 1
                    if s == 0:
                        self.a2cur = sb.tile([100, 2, N], FP8, tag="a2",
                                             bufs=cfg["a2_bufs"],
                                             name=f"a2_{nt}_{yo}")
                    nc.scalar.activation(self.a2cur[:, s, :], p2[:], Relu,
                                         bias=b2col[s], scale=E12)
                    self.ei += 1
                    if s == 1:
                        self.pending.append((self.a2cur, yo))
                        if len(self.pending) > cfg["pending_lag"]:
                            self._fc1()

                def _fc1(self, last=False):
                    pa2, yo = self.pending.pop(0)
                    co = yo * W8_YO
                    nc.tensor.matmul(
                        self.pfc1[:], wb8[0:100, co:co + W8_YO].rearrange(
                            "p (two m) -> p two m", two=2),
                        pa2[:], start=(self.gi == 0), stop=last,
                        perf_mode=DR)
                    self.gi += 1

                def flush(self):
                    while self.pending:
                        self._fc1(last=(len(self.pending) == 1))
                    return self.pfc1

            def tail_stage(nt, pfc1):
                """fc2 + log_softmax + output DMA, pipelined in chunks so
                the serial exp/ln chain isn't fully exposed."""
                nh = cfg["tail_halves"]
                NH = N // nh
                a3 = sb.tile([50, N], F32R, tag="a3", bufs=2,
                             name=f"a3_{nt}")
                for hx in range(nh):
                    c0, c1 = hx * NH, (hx + 1) * NH
                    n0 = nt * N + c0
                    nc.scalar.activation(a3[:, c0:c1], pfc1[0:50, c0:c1],
                                         Relu, bias=bf1col, scale=1.0 / E12)
                    ptag = "ptail" if hx % 2 == 0 else "pfc1"
                    zps = ps.tile([10, NH], F32, tag=ptag, bufs=1,
                                  name=f"zps_{nt}_{hx}")
                    nc.tensor.matmul(zps[:],
                                     wr(0, 50, C_LHSTF2, C_LHSTF2 + 10),
                                     a3[:, c0:c1], start=True, stop=True)
                    z = sb.tile([10, NH], F32, tag="z", bufs=2,
                                name=f"z_{nt}_{hx}")
                    nc.vector.tensor_scalar(z[:], zps[:], Etot, bf2col,
                                            Mult, Add)
                    ez = sb.tile([10, NH], F32R, tag="ez", bufs=2,
                                 name=f"ez_{nt}_{hx}")
                    nc.scalar.activation(ez[:], zps[:], Exp, bias=bf2col,
                                         scale=Etot)
                    sps = ps.tile([1, NH], F32, tag=ptag, bufs=1,
                                  name=f"sps_{nt}_{hx}")
                    nc.tensor.matmul(sps[:],
                                     wr(0, 10, C_ONES_COL, C_ONES_COL + 1),
                                     ez[:], start=True, stop=True)
                    lse = sb.tile([1, NH], F32R, tag="lse", bufs=2,
                                  name=f"lse_{nt}_{hx}")
                    nc.scalar.activation(lse[:], sps[:], Ln)
                    bps = ps.tile([10, NH], F32, tag=ptag, bufs=1,
                                  name=f"bps_{nt}_{hx}")
                    nc.tensor.matmul(bps[:],
                                     wr(0, 1, C_ONES_ROW, C_ONES_ROW + 10),
                                     lse[:], start=True, stop=True)
                    osb = sb.tile([10, NH], F32, tag="osb", bufs=2,
                                  name=f"osb_{nt}_{hx}")
                    nc.vector.tensor_sub(osb[:], z[:], bps[:])
                    nc.sync.dma_start(out[:, n0:n0 + NH], osb[:])

            # explicit fine-grained interleave: each conv1 group of tile
            # nt is followed by ~2 conv2 groups of tile nt-1, so the PE has
            # dense matmul work while the pool chains drain on ACT/DVE
            for _rep in range(repeat):
                prev = None  # Conv2State of tile nt-1
                for nt in range(n_tiles):
                    r2 = alloc_r2(nt)
                    ci = 0
                    for g in range(12):
                        t, h = divmod(g, 2)
                        conv1_group(nt, t, h, r2, g)
                        if prev is not None:
                            want = 20 * (g + 1) // 12
                            while ci < want:
                                prev.group()
                                ci += 1
                    if prev is not None:
                        tail_stage(prev.nt, prev.flush())
                    prev = Conv2State(nt, r2)
                while prev.ei < 20:
                    prev.group()
                tail_stage(prev.nt, prev.flush())
    nc.compile()
    return nc


def kernel(**inputs):
    global LAST_EXEC_TIME_NS, LAST_RESULTS
    x = np.ascontiguousarray(np.asarray(inputs["x"], dtype=np.float32))
    wb16, wb, wb8, scales = _host_prep(
        {k: np.asarray(v) for k, v in inputs.items()})

    nc = build_program(scales)

    import ml_dtypes
    wb16_b = wb16.astype(ml_dtypes.bfloat16)
    wb8_b = wb8.astype(ml_dtypes.float8_e4m3)
    in_maps = []
    for i in range(N_CORES):
        xs = x[i * BC:(i + 1) * BC, 0]            # [BC, 28, 28]
        xTi = np.ascontiguousarray(
            xs.transpose(1, 2, 0).astype(ml_dtypes.bfloat16))  # [28, 28, BC]
        in_maps.append({"xT": xTi, "wblob16": wb16_b, "wblob": wb,
                        "wblob8": wb8_b})

    trace = bool(os.environ.get("KERNEL_TRACE"))
    res = run_bass_kernel_spmd(nc, in_maps, list(range(N_CORES)), trace=trace)
    LAST_EXEC_TIME_NS = res.exec_time_ns
    LAST_RESULTS = res

    out = np.empty((B_TOTAL, 10), np.float32)
    for i in range(N_CORES):
        out[i * BC:(i + 1) * BC] = res.results[i]["out"].T
    return out
